# revision 67
# baseline (speedup 1.0000x reference)
"""Trainium2 Bass kernel for nn_AdaLNConditioning (HGRNBitMLP + AdaLN head).

Strategy:
- Data-parallel over tokens: 8192 tokens -> 1024 per core, no collectives.
- Host precomputes ternary weight quantization (BitNet b1.58 global-mean
  scale) and packs transposed weight tiles in streaming order as fp8e4
  (ternary {-1,0,1} is exact in e4m3; PE mixed bf16 x fp8 matmul verified
  bit-exact on HW). Halves weight HBM traffic and DGE descriptor count
  vs bf16.
- On device, per token tile [128, D]: RMSNorm stats + per-token int8
  quantization (round-to-nearest-even via the 1.5*2^23 magic constant,
  bit-exact with jnp.round), quantized codes stored as bf16 (integers
  <= 127 are exact in bf16), transposed into [K, token] layout with PE
  transposes batched 8-per-PSUM-bank + one DVE copy-out in the prologue
  (no DGE dependency) and DMA-xbar transposes on the sync queue in
  steady state.
- Matmuls run on bf16 codes x fp8 weights with f32 PSUM accumulation ->
  exact integer arithmetic; per-token dequant scale applied at PSUM
  evacuation (fused into ScalarE/VectorE copy). The bf16 N=512 matmul
  stream is the PE roofline for this problem: fp8 DoubleRow fails
  accuracy (e4m3 activations ~2.8%/layer vs the 2e-2 budget; an exact
  hi/lo split costs 2 DoubleRow matmuls = 1.39x bf16), and uint8 matmul
  is rejected by walrus codegen (s3d3_mm_dtype ISA check).
- Each layer is processed in NP=4 pieces of TP=2 token tiles (rather
  than 2 halves of 4): the first piece's quant is the only exposed
  (PE-idle) latency, so halving the piece cuts the prologue roughly in
  half. Weights stream once per piece (2x the traffic of the half
  schedule) which still fits the DMA budget when split across queues.
- swiglu intermediate z and down-proj output h round-trip through HBM
  in f32 (bf16 storage costs ~1.5e-2 rel err; f32 keeps e2e ~1.4e-3).
- DMA queues (only SP/Activation/gpsimd can initiate DMAs): sync DGE =
  code transposes only (the scalar-queue xbar path corrupts data, so
  sync is the only valid transpose queue); scalar DGE = weight
  half-chunk 0 + activation reads (x/z/h); gpsimd DGE = weight
  half-chunk 1 + z/h/out writes. Prologue piece 0 splits its x loads
  across sync+scalar (both otherwise idle there).
- norm_weight is all-ones for this module (checked on host): the AdaLN
  RMSNorm folds into the L3 bit_linear renorm as a pure [P,1] scalar
  chain on the existing ssq stats; no nw broadcast or second stats
  pass. A general nw path is kept as fallback.
"""

import sys
from contextlib import ExitStack

import numpy as np
import ml_dtypes

sys.path.insert(0, "/opt/trn_rl_repo")

import concourse.bass as bass  # noqa: E402
import concourse.tile as tile  # noqa: E402
from concourse import bacc  # noqa: E402
from concourse import mybir  # noqa: E402
from concourse.masks import make_identity  # noqa: E402

AF = mybir.ActivationFunctionType
ALU = mybir.AluOpType
F32 = mybir.dt.float32
BF16 = mybir.dt.bfloat16
FP8 = mybir.dt.float8e4

P = 128
MAGIC = 12582912.0  # 1.5 * 2**23: add+store rounds f32 to nearest-even integer
N_CORES = 8


class Cfg:
    def __init__(self, T=1024, D=4096, INTER=4096, CW=512, TP=2):
        self.T = T            # tokens per core
        self.D = D            # model dim (k of L1/L3, out of L2/L3)
        self.INTER = INTER    # swiglu intermediate
        self.CW = CW          # output-chunk width (matmul moving free dim)
        self.TT = T // P      # token tiles per core
        self.TP = TP          # token tiles per piece
        self.NP = self.TT // TP
        self.GCH = 2 * INTER // CW  # L1 chunks (v/gate interleaved)
        self.DCH = D // CW          # L2/L3 chunks
        self.KT1 = D // P
        self.KT2 = INTER // P


def host_weight_quant(w):
    """BitNet ternary quant. Returns (codes {-1,0,1} f32, scale) matching
    jnp: scale = 1/clip(mean|w|, 1e-5); q = clip(round(w*scale), -1, 1)."""
    mean_abs = np.mean(np.abs(w), dtype=np.float64).astype(np.float32)
    s = np.float32(1.0) / np.maximum(mean_abs, np.float32(1e-5))
    q = np.clip(np.round(w * s), -1, 1).astype(np.float32)
    return q, s


def pack_weight(WqT, col_starts, cfg):
    """Pack WqT [K, O] into [n_chunks, 2, P, KT/2, CW] fp8 half-chunk
    streaming layout: per chunk, two DMA-able halves, each with 8KB of
    contiguous per-partition data (k-tile-major within the half)."""
    K = WqT.shape[0]
    KT = K // P
    KH = KT // 2
    out = np.empty((len(col_starts), 2, P, KH, cfg.CW), dtype=ml_dtypes.float8_e4m3fn)
    for ci, c0 in enumerate(col_starts):
        blk = WqT[:, c0:c0 + cfg.CW]                       # [K, CW]
        # row k = kt*P + p -> [hc, p, kh, cw] with kt = hc*KH + kh
        blk = blk.reshape(2, KH, P, cfg.CW).transpose(0, 2, 1, 3)
        out[ci] = blk.astype(ml_dtypes.float8_e4m3fn)
    return out


def build_nc(cfg, sg, sd, so, nw_ones=True):
    """Build the single-core (SPMD) Bass program."""
    nc = bacc.Bacc()
    T, D, INTER, CW = cfg.T, cfg.D, cfg.INTER, cfg.CW
    TP, NP = cfg.TP, cfg.NP
    KT1, KT2, GCH, DCH = cfg.KT1, cfg.KT2, cfg.GCH, cfg.DCH
    QW = min(1024, D)             # quant sub-chunk width
    KTQ = QW // P                 # k-tiles per quant sub-chunk (psum slab)

    x_p = nc.declare_dram_parameter("x", [T, D], F32, isOutput=False)
    wg_p = nc.declare_dram_parameter("wg", [GCH, 2, P, KT1 // 2, CW], FP8, isOutput=False)
    wd_p = nc.declare_dram_parameter("wd", [DCH, 2, P, KT2 // 2, CW], FP8, isOutput=False)
    wo_p = nc.declare_dram_parameter("wo", [DCH, 2, P, KT1 // 2, CW], FP8, isOutput=False)
    if not nw_ones:
        nw_p = nc.declare_dram_parameter("nw", [1, D], F32, isOutput=False)
    out_p = nc.declare_dram_parameter("out", [T, D], F32, isOutput=True)

    c_gate = float(1.0 / (127.0 * sg))
    c_down = float(1.0 / (127.0 * sd))
    c_out = float(1.0 / (127.0 * so))

    with ExitStack() as ctx:
        tc = ctx.enter_context(tile.TileContext(nc))
        singles = ctx.enter_context(tc.tile_pool(name="singles", bufs=1))
        small = ctx.enter_context(tc.tile_pool(name="small", bufs=96))
        xin = ctx.enter_context(tc.tile_pool(name="xin", bufs=2))      # [P,QW] f32
        rts = ctx.enter_context(tc.tile_pool(name="rts", bufs=4))      # [P,QW] f32 scratch
        qt_pool = ctx.enter_context(tc.tile_pool(name="qt", bufs=2))   # [P,KT,TP*P] bf16
        wpool = ctx.enter_context(tc.tile_pool(name="wpool", bufs=8))  # [P,KT/2,CW] fp8
        gv = ctx.enter_context(tc.tile_pool(name="gv", bufs=2))        # [P,TP,CW] f32 per tag
        zpool = ctx.enter_context(tc.tile_pool(name="zpool", bufs=2))  # [P,TP,CW] f32
        mm_ps = ctx.enter_context(tc.tile_pool(name="mmps", bufs=6, space="PSUM"))
        tp_ps = ctx.enter_context(tc.tile_pool(name="tpps", bufs=2, space="PSUM"))
        dram = ctx.enter_context(tc.tile_pool(name="dram", bufs=1, space="DRAM"))

        eps_t = {}
        for ev in (1e-8, 1e-6):
            et = singles.tile([P, 1], F32, name=f"eps{ev}")
            nc.vector.memset(et, ev)
            eps_t[ev] = et
        ident = singles.tile([P, P], BF16, name="ident")
        make_identity(nc, ident)
        if not nw_ones:
            nw_bc = singles.tile([P, D], F32)
            nw_ap = nw_p[:]
            nc.sync.dma_start(
                out=nw_bc,
                in_=bass.AP(tensor=nw_ap.tensor, offset=nw_ap.offset, ap=[[0, P], [1, D]]),
            )

        def reduce_cols(parts, fn):
            """Combine [P,1] tiles with a binary DVE op; returns final tile."""
            while len(parts) > 1:
                nxt = []
                for i in range(0, len(parts) - 1, 2):
                    o = small.tile([P, 1], F32, tag="s", name="comb")
                    fn(o, parts[i], parts[i + 1])
                    nxt.append(o)
                if len(parts) % 2:
                    nxt.append(parts[-1])
                parts = nxt
            return parts[0]

        def quant_gen(src_ap, KTn, base_tt, qTt, c_t, c_const, eps, nw=False,
                      nw_eps=None, pe_tp=False, fine_tp_yield=False):
            """Norm + int8-quant + transpose for TP token tiles of one piece.

            Quant scale is qs = 127/absmax(t2) (the rsqrt factor cancels
            algebraically between quant and dequant); the dequant scale
            c = (am*c_const)*r carries the norm factor r off the critical
            path. rt = (t2*qs + MAGIC) rounds to integer+MAGIC at the f32
            store; the -MAGIC subtract produces bf16 codes (exact for ints
            <= 127), then transposes move them into [K, token] layout.

            nw path: reference computes h1 = h*rsqrt(mean h^2 + nw_eps)*nw,
            then bit_linear renorms h2 = h1*rsqrt(mean h1^2 + eps); both
            fold into one per-token factor r = r1*r2 on (h*nw).  With
            nw == 1 the codes equal the plain path's and
            mean((h*r1)^2) == r1^2*ssq/DL, so the second elementwise stats
            pass collapses into [P,1] ops on ssq.
            """
            DL = KTn * P
            NQ = DL // QW
            for i in range(TP):
                tt = base_tt + i
                x_js = []
                for j in range(NQ):
                    x_j = xin.tile([P, QW], F32, tag="xin", bufs=10, name="xj")
                    # prologue: sync+scalar queues are otherwise idle, so
                    # split the x bolus across them; steady state: scalar
                    # queue (transposes own sync, writes own gpsimd)
                    if pe_tp:
                        eng = nc.sync if j % 2 == 1 else nc.scalar
                    else:
                        eng = nc.scalar
                    eng.dma_start(
                        out=x_j,
                        in_=src_ap[tt * P:(tt + 1) * P, j * QW:(j + 1) * QW])
                    x_js.append(x_j)
                sparts = []
                for j in range(NQ):
                    so_ = rts.tile([P, QW], F32, tag="sq", bufs=2)
                    ssj = small.tile([P, 1], F32, tag="s", name="ssj")
                    nc.scalar.activation(so_, x_js[j], AF.Square,
                                         accum_out=ssj)
                    sparts.append(ssj)
                ssq = reduce_cols(sparts, nc.vector.tensor_add)
                r = small.tile([P, 1], F32, tag="s")
                aparts = []
                if not nw:
                    std = small.tile([P, 1], F32, tag="s")
                    nc.scalar.activation(std, ssq, AF.Sqrt, scale=1.0 / DL, bias=eps_t[eps])
                    nc.vector.reciprocal(r, std)
                    for j in range(NQ):
                        amj = small.tile([P, 1], F32, tag="s", name="amj")
                        nc.vector.tensor_reduce(amj, x_js[j],
                                                axis=mybir.AxisListType.X, op=ALU.max,
                                                apply_absolute_value=True)
                        aparts.append(amj)
                else:
                    std1 = small.tile([P, 1], F32, tag="s")
                    nc.scalar.activation(std1, ssq, AF.Sqrt, scale=1.0 / DL,
                                         bias=eps_t[nw_eps])
                    r1 = small.tile([P, 1], F32, tag="s")
                    nc.vector.reciprocal(r1, std1)
                    if nw_ones:
                        ssq2 = ssq
                        for j in range(NQ):
                            amj = small.tile([P, 1], F32, tag="s", name="amj")
                            nc.vector.tensor_reduce(amj, x_js[j],
                                                    axis=mybir.AxisListType.X,
                                                    op=ALU.max,
                                                    apply_absolute_value=True)
                            aparts.append(amj)
                    else:
                        s2parts = []
                        for j in range(NQ):
                            t2j = rts.tile([P, QW], F32, tag="t2", bufs=1)
                            nc.vector.tensor_mul(t2j, x_js[j],
                                                 nw_bc[:, j * QW:(j + 1) * QW])
                            so2 = rts.tile([P, QW], F32, tag="sq", bufs=1)
                            ss2j = small.tile([P, 1], F32, tag="s", name="ss2j")
                            nc.scalar.activation(so2, t2j, AF.Square, accum_out=ss2j)
                            s2parts.append(ss2j)
                            amj = small.tile([P, 1], F32, tag="s", name="amj")
                            nc.vector.tensor_reduce(amj, t2j, axis=mybir.AxisListType.X,
                                                    op=ALU.max, apply_absolute_value=True)
                            aparts.append(amj)
                        ssq2 = reduce_cols(s2parts, nc.vector.tensor_add)
                    u = small.tile([P, 1], F32, tag="s")
                    nc.vector.tensor_mul(u, r1, r1)
                    w2 = small.tile([P, 1], F32, tag="s")
                    nc.vector.tensor_mul(w2, u, ssq2)
                    std2 = small.tile([P, 1], F32, tag="s")
                    nc.scalar.activation(std2, w2, AF.Sqrt, scale=1.0 / DL, bias=eps_t[eps])
                    r2 = small.tile([P, 1], F32, tag="s")
                    nc.vector.reciprocal(r2, std2)
                    nc.vector.tensor_mul(r, r1, r2)
                am = reduce_cols(aparts, nc.vector.tensor_max)
                invam = small.tile([P, 1], F32, tag="s")
                nc.vector.reciprocal(invam, am)
                qs = small.tile([P, 1], F32, tag="s")
                nc.vector.tensor_scalar_mul(qs, invam, 127.0)
                nc.vector.scalar_tensor_tensor(c_t[:, i:i + 1], am, c_const, r,
                                               op0=ALU.mult, op1=ALU.mult)
                # rt = t2*qs + MAGIC (f32 store rounds to nearest-even int)
                rtjs = []
                for j in range(NQ):
                    if not nw or nw_ones:
                        src_j = x_js[j]
                    else:
                        src_j = rts.tile([P, QW], F32, tag="t2", bufs=1)
                        nc.vector.tensor_mul(src_j, x_js[j],
                                             nw_bc[:, j * QW:(j + 1) * QW])
                    rt_j = rts.tile([P, QW], F32, tag="rt", bufs=4)
                    nc.vector.tensor_scalar(rt_j, src_j, scalar1=qs, scalar2=MAGIC,
                                            op0=ALU.mult, op1=ALU.add)
                    rtjs.append(rt_j)
                # subtract MAGIC -> bf16 codes, then transpose into qT.
                # Prologue piece (pe_tp): PE transposes batched 8-per-PSUM
                # bank + one DVE copy-out (short latency chain, no DGE).
                # Steady-state pieces: DMA-xbar transposes on the sync DGE
                # queue, which they get to themselves (the scalar-queue
                # xbar path corrupts data, so sync is the only valid
                # transpose queue).
                # qb stays on vector for BOTH parities: a scalar-engine qb
                # waits on the transpose that frees its buffer, and that
                # head-of-line blocks evac ACTIVATEs for 10s of us at piece
                # boundaries. bufs=8 reaches a full piece back so the
                # recycle wait is always long-satisfied.
                for j in range(NQ):
                    q_j = rts.tile([P, QW], BF16, tag="qb", bufs=8)
                    nc.vector.tensor_scalar_add(q_j, rtjs[j], -MAGIC)
                    dst = qTt[:, j * KTQ:(j + 1) * KTQ, i * P:(i + 1) * P]
                    # PE transposes only for the very first token tile: tile
                    # 1's PE transposes would sit in the PE FIFO ahead of the
                    # first product matmuls (which only need tile 0's codes)
                    # and idle the PE ~10us; tile 1 rides the sync xbar and
                    # overlaps the first matmul stream instead.
                    if pe_tp and i == 0:
                        tp = tp_ps.tile([P, KTQ, P], BF16, tag="tp")
                        for b in range(KTQ):
                            nc.tensor.transpose(
                                tp[:, b, :], q_j[:, b * P:(b + 1) * P], ident
                            )
                        nc.vector.tensor_copy(dst, tp)
                        # fine-grained yield: the first chunk's matmul
                        # kt-groups are emitted between these transpose
                        # batches so the PE starts multiplying after batch
                        # j0 instead of waiting for all 32 transposes
                        if fine_tp_yield:
                            yield
                    else:
                        nc.sync.dma_start_transpose(dst, q_j)
                if not (fine_tp_yield and i == 0):
                    yield

        def mm_gen(w_p, nch, KTn, qTt, evac, qg_interleave=None):
            KH = KTn // 2
            ii = list(range(TP))
            for c in range(nch):
                wts = []
                for g in range(2):
                    wt = wpool.tile([P, KH, CW], FP8, tag="w")
                    weng = nc.scalar if g == 0 else nc.gpsimd
                    weng.dma_start(out=wt, in_=w_p[c, g])
                    wts.append(wt)
                for i in ii:
                    ps = mm_ps.tile([P, CW], F32, tag="mm")
                    if c == 0 and i == 0 and qg_interleave is not None:
                        # first chunk rides the transpose batches: emit each
                        # kt-group right after the tp batch that wrote it
                        # (driver pre-emitted batch j0), keeping emission
                        # order = dependency order
                        for g4 in range(KTn // KTQ):
                            if g4 > 0:
                                next(qg_interleave, None)
                            for kt in range(g4 * KTQ, (g4 + 1) * KTQ):
                                rhs = wts[kt // KH][:, kt % KH, :]
                                nc.tensor.matmul(
                                    ps,
                                    lhsT=qTt[:, kt, i * P:(i + 1) * P],
                                    rhs=rhs,
                                    start=(kt == 0),
                                    stop=(kt == KTn - 1),
                                )
                        # tile 1's quant emission, before any i=1 matmul
                        next(qg_interleave, None)
                        qg_interleave = None
                    else:
                        for kt in range(KTn):
                            rhs = wts[kt // KH][:, kt % KH, :]
                            nc.tensor.matmul(
                                ps,
                                lhsT=qTt[:, kt, i * P:(i + 1) * P],
                                rhs=rhs,
                                start=(kt == 0),
                                stop=(kt == KTn - 1),
                            )
                    evac.step(c, i, ps)  # psum bank freed per i-group
                evac.flush(c, ii)
                yield

        # ---- dram intermediates (per piece: no false cross-piece deps) ----
        z_ds = [dram.tile([T // NP, INTER], F32, name=f"z{p}", tag=f"z{p}")
                for p in range(NP)]
        z_rs = [zd[:].rearrange("(a p) n -> p a n", p=P) for zd in z_ds]
        h_ds = [dram.tile([T // NP, D], F32, name=f"h{p}", tag=f"h{p}")
                for p in range(NP)]
        h_rs = [hd[:].rearrange("(a p) n -> p a n", p=P) for hd in h_ds]
        out_r = out_p[:].rearrange("(a p) n -> p a n", p=P)
        st = {}

        class Evac1:
            # chunk order v0,g0,v1,g1,...: v dequantized x c1^2 (extra c1
            # pre-applies gate dequant); z = (psum_g * sigmoid(psum_g*c1)) * v
            def __init__(self, c1h, piece):
                self.c1h = c1h
                self.piece = piece

            def step(self, c, i, ps):
                cc = self.c1h[:, i:i + 1]
                if c % 2 == 0:
                    if st.get("v_new", True):
                        st["v"] = gv.tile([P, TP, CW], F32, tag="v", name="v_t")
                        st["v_new"] = False
                    nc.vector.tensor_scalar(st["v"][:, i, :], ps, scalar1=cc,
                                            scalar2=cc, op0=ALU.mult, op1=ALU.mult)
                else:
                    if st.get("sig_new", True):
                        st["sig"] = gv.tile([P, TP, CW], F32, tag="sig", bufs=1, name="sig_t")
                        st["z"] = zpool.tile([P, TP, CW], F32, tag="z", name="z_t")
                        st["sig_new"] = False
                    nc.scalar.activation(st["sig"][:, i, :], ps, AF.Sigmoid, scale=cc)
                    nc.vector.tensor_mul(st["z"][:, i, :], ps, st["sig"][:, i, :])

            def flush(self, c, ii):
                if c % 2 == 0:
                    st["sig_new"] = True
                else:
                    st["v_new"] = True
                    lo, hi = ii[0], ii[-1] + 1
                    z_t = st["z"]
                    nc.vector.tensor_mul(z_t[:, lo:hi, :], z_t[:, lo:hi, :],
                                         st["v"][:, lo:hi, :])
                    nc.gpsimd.dma_start(
                        out=z_rs[self.piece][:, lo:hi, (c // 2) * CW:(c // 2 + 1) * CW],
                        in_=z_t[:, lo:hi, :]
                    )

        class EvacPlain:
            def __init__(self, c_th, dst_r, row0, final=False):
                self.c_th = c_th
                self.dst_r = dst_r
                self.row0 = row0
                self.final = final

            def step(self, c, i, ps):
                if st.get("o_new", True):
                    st["o"] = zpool.tile([P, TP, CW], F32, tag="z", name="o_t")
                    st["o_new"] = False
                cc = self.c_th[:, i:i + 1]
                if i % 2 == 0:
                    nc.vector.tensor_scalar(st["o"][:, i, :], ps, scalar1=cc,
                                            scalar2=None, op0=ALU.mult)
                else:
                    nc.scalar.activation(st["o"][:, i, :], ps, AF.Copy, scale=cc)

            def flush(self, c, ii):
                st["o_new"] = True
                if self.final:
                    # last piece: per-i writes on alternating queues so the
                    # post-last-matmul DMA drain is ~1 write, not a backlog
                    for i in ii:
                        eng = nc.gpsimd if i % 2 == 0 else nc.sync
                        eng.dma_start(
                            out=self.dst_r[:, self.row0 + i:self.row0 + i + 1,
                                           c * CW:(c + 1) * CW],
                            in_=st["o"][:, i:i + 1, :]
                        )
                    return
                lo, hi = ii[0], ii[-1] + 1
                row0 = self.row0 + lo
                nc.gpsimd.dma_start(
                    out=self.dst_r[:, row0:row0 + hi - lo, c * CW:(c + 1) * CW],
                    in_=st["o"][:, lo:hi, :]
                )

        # ---- pipelined stages: emission INTERLEAVED so quant(stage k+1)
        # overlaps mm(stage k) on every engine's instruction stream ----
        def stage_factory(L, p):
            def mk():
                ct = singles.tile([P, TP], F32, name=f"c{L}_{p}")
                if L == 1:
                    qT = qt_pool.tile([P, KT1, TP * P], BF16, tag="qt")
                    qg = quant_gen(x_p[:], KT1, p * TP, qT, ct, c_gate, 1e-8,
                                   pe_tp=(p == 0), fine_tp_yield=(p == 0))
                    mmf = lambda: mm_gen(wg_p, GCH, KT1, qT, Evac1(ct, p),
                                         qg_interleave=(qg if p == 0 else None))
                    return qg, mmf
                if L == 2:
                    qT = qt_pool.tile([P, KT2, TP * P], BF16, tag="qt")
                    qg = quant_gen(z_ds[p][:], KT2, 0, qT, ct, c_down, 1e-8)
                    mmf = lambda: mm_gen(wd_p, DCH, KT2, qT,
                                         EvacPlain(ct, h_rs[p], 0))
                    return qg, mmf
                qT = qt_pool.tile([P, KT1, TP * P], BF16, tag="qt")
                qg = quant_gen(h_ds[p][:], KT1, 0, qT, ct, c_out, 1e-8,
                               nw=True, nw_eps=1e-6)
                mmf = lambda: mm_gen(wo_p, DCH, KT1, qT,
                                     EvacPlain(ct, out_r, p * TP,
                                               final=(p == NP - 1)))
                return qg, mmf
            return mk

        stage_mks = [stage_factory(L, p) for L in (1, 2, 3) for p in range(NP)]
        qg0, mmf = stage_mks[0]()
        next(qg0, None)  # tile-0 chain through transpose batch j0; the rest
        #                  interleaves into the first matmul chunk (mm_gen)
        for k in range(len(stage_mks)):
            mm = mmf()
            if k + 1 < len(stage_mks):
                qn, mmf = stage_mks[k + 1]()
            else:
                qn = None
            # Emit the ENTIRE next-stage quant right after the first chunk:
            # gradual interleave leaves the quant tail sandwiched behind
            # psum-gated evac ops at the end of each engine FIFO, which
            # re-creates a ~13us late-quant equilibrium at every piece
            # boundary. Early full emission lets it run at piece start.
            # (A reader emitted before its writer does NOT get a dependency
            # — emission order defines the DAG — so the quant must be fully
            # emitted before any matmul that reads its codes.)
            ci = 0
            for _ in mm:
                ci += 1
                if qn is not None and ci >= 1:
                    for _ in qn:
                        pass
                    qn = None
    return nc


def prepare_inputs(condition, w_gate, w_down, norm_weight, w_out, cfg, n_cores=N_CORES):
    """Host-side: quantize+pack weights, shard tokens. Returns (in_maps, scales)."""
    TOK = condition.shape[0] * condition.shape[1]
    X = np.ascontiguousarray(condition.reshape(TOK, cfg.D).astype(np.float32, copy=False))

    Wg, sg = host_weight_quant(np.asarray(w_gate, dtype=np.float32))
    Wd, sd = host_weight_quant(np.asarray(w_down, dtype=np.float32))
    Wo, so = host_weight_quant(np.asarray(w_out, dtype=np.float32))

    # L1 chunk order interleaves v/gate so swiglu can fuse per chunk pair
    l1_cols = []
    for i in range(cfg.INTER // cfg.CW):
        l1_cols += [cfg.INTER + i * cfg.CW, i * cfg.CW]
    WG = pack_weight(Wg.T, l1_cols, cfg)
    WD = pack_weight(Wd.T, [i * cfg.CW for i in range(cfg.D // cfg.CW)], cfg)
    WO = pack_weight(Wo.T, [i * cfg.CW for i in range(cfg.D // cfg.CW)], cfg)

    nw = np.ascontiguousarray(np.asarray(norm_weight, dtype=np.float32).reshape(1, cfg.D))
    nw_ones = bool(np.all(nw == np.float32(1.0)))

    in_maps = []
    for i in range(n_cores):
        m = {
            "x": np.ascontiguousarray(X[i * cfg.T:(i + 1) * cfg.T]),
            "wg": WG, "wd": WD, "wo": WO,
        }
        if not nw_ones:
            m["nw"] = nw
        in_maps.append(m)
    return in_maps, (sg, sd, so), nw_ones


def run(condition, w_gate, w_down, norm_weight, w_out, cfg=None, trace=False, tmpdir=None):
    from concourse.bass_utils import run_bass_kernel_spmd
    if cfg is None:
        cfg = Cfg()
    in_maps, (sg, sd, so), nw_ones = prepare_inputs(condition, w_gate, w_down,
                                                     norm_weight, w_out, cfg)
    nc = build_nc(cfg, sg, sd, so, nw_ones=nw_ones)
    nc.finalize()
    # transient NRT_EXEC_UNIT_UNRECOVERABLE device crashes recover on retry
    last_err = None
    for attempt in range(3):
        try:
            res = run_bass_kernel_spmd(nc, in_maps, list(range(N_CORES)), trace=trace,
                                       tmpdir=tmpdir)
            break
        except Exception as e:  # noqa: BLE001
            last_err = e
            if attempt == 2:
                raise
            import time as _time
            _time.sleep(20)
    outs = np.concatenate([np.asarray(res.results[i]["out"]) for i in range(N_CORES)], axis=0)
    B, S = condition.shape[0], condition.shape[1]
    Pfull = outs.reshape(B, S, cfg.D)
    H = cfg.D // 2
    return (Pfull[..., :H], Pfull[..., H:]), res


def kernel(condition, w_gate, w_down, norm_weight, w_out):
    (scale, shift), _ = run(condition, w_gate, w_down, norm_weight, w_out)
    return scale, shift


# revision 68
# speedup vs baseline: 1.1938x; 1.1938x over previous
"""Trainium2 Bass kernel for nn_AdaLNConditioning (HGRNBitMLP + AdaLN head).

Strategy:
- Data-parallel over tokens: 8192 tokens -> 1024 per core, no collectives.
- Host precomputes ternary weight quantization (BitNet b1.58 global-mean
  scale) and packs transposed weight tiles in streaming order as fp8e4
  (ternary {-1,0,1} is exact in e4m3; PE mixed bf16 x fp8 matmul verified
  bit-exact on HW). Halves weight HBM traffic and DGE descriptor count
  vs bf16.
- On device, per token tile [128, D]: RMSNorm stats + per-token int8
  quantization (round-to-nearest-even via the 1.5*2^23 magic constant,
  bit-exact with jnp.round), quantized codes stored as bf16 (integers
  <= 127 are exact in bf16), transposed into [K, token] layout with PE
  transposes batched 8-per-PSUM-bank + one DVE copy-out in the prologue
  (no DGE dependency) and DMA-xbar transposes on the sync queue in
  steady state.
- Matmuls run on bf16 codes x fp8 weights with f32 PSUM accumulation ->
  exact integer arithmetic; per-token dequant scale applied at PSUM
  evacuation (fused into ScalarE/VectorE copy). The bf16 N=512 matmul
  stream is the PE roofline for this problem: fp8 DoubleRow fails
  accuracy (e4m3 activations ~2.8%/layer vs the 2e-2 budget; an exact
  hi/lo split costs 2 DoubleRow matmuls = 1.39x bf16), and uint8 matmul
  is rejected by walrus codegen (s3d3_mm_dtype ISA check).
- Each layer is processed in NP=4 pieces of TP=2 token tiles (rather
  than 2 halves of 4): the first piece's quant is the only exposed
  (PE-idle) latency, so halving the piece cuts the prologue roughly in
  half. Weights stream once per piece (2x the traffic of the half
  schedule) which still fits the DMA budget when split across queues.
- swiglu intermediate z and down-proj output h round-trip through HBM
  in f32 (bf16 storage costs ~1.5e-2 rel err; f32 keeps e2e ~1.4e-3).
- DMA queues (only SP/Activation/gpsimd can initiate DMAs): sync DGE =
  code transposes only (the scalar-queue xbar path corrupts data, so
  sync is the only valid transpose queue); scalar DGE = weight
  half-chunk 0 + activation reads (x/z/h); gpsimd DGE = weight
  half-chunk 1 + z/h/out writes. Prologue piece 0 splits its x loads
  across sync+scalar (both otherwise idle there).
- norm_weight is all-ones for this module (checked on host): the AdaLN
  RMSNorm folds into the L3 bit_linear renorm as a pure [P,1] scalar
  chain on the existing ssq stats; no nw broadcast or second stats
  pass. A general nw path is kept as fallback.
"""

import sys
from contextlib import ExitStack

import numpy as np
import ml_dtypes

sys.path.insert(0, "/opt/trn_rl_repo")

import concourse.bass as bass  # noqa: E402
import concourse.tile as tile  # noqa: E402
from concourse import bacc  # noqa: E402
from concourse import mybir  # noqa: E402
from concourse.masks import make_identity  # noqa: E402

AF = mybir.ActivationFunctionType
ALU = mybir.AluOpType
F32 = mybir.dt.float32
BF16 = mybir.dt.bfloat16
FP8 = mybir.dt.float8e4

P = 128
MAGIC = 12582912.0  # 1.5 * 2**23: add+store rounds f32 to nearest-even integer
N_CORES = 8


class Cfg:
    def __init__(self, T=1024, D=4096, INTER=4096, CW=512, TP=2):
        self.T = T            # tokens per core
        self.D = D            # model dim (k of L1/L3, out of L2/L3)
        self.INTER = INTER    # swiglu intermediate
        self.CW = CW          # output-chunk width (matmul moving free dim)
        self.TT = T // P      # token tiles per core
        self.TP = TP          # token tiles per piece
        self.NP = self.TT // TP
        self.GCH = 2 * INTER // CW  # L1 chunks (v/gate interleaved)
        self.DCH = D // CW          # L2/L3 chunks
        self.KT1 = D // P
        self.KT2 = INTER // P


def host_weight_quant(w):
    """BitNet ternary quant. Returns (codes {-1,0,1} f32, scale) matching
    jnp: scale = 1/clip(mean|w|, 1e-5); q = clip(round(w*scale), -1, 1)."""
    mean_abs = np.mean(np.abs(w), dtype=np.float64).astype(np.float32)
    s = np.float32(1.0) / np.maximum(mean_abs, np.float32(1e-5))
    q = np.clip(np.round(w * s), -1, 1).astype(np.float32)
    return q, s


def pack_weight(WqT, col_starts, cfg):
    """Pack WqT [K, O] into [n_chunks, 2, P, KT/2, CW] fp8 half-chunk
    streaming layout: per chunk, two DMA-able halves, each with 8KB of
    contiguous per-partition data (k-tile-major within the half)."""
    K = WqT.shape[0]
    KT = K // P
    KH = KT // 2
    out = np.empty((len(col_starts), 2, P, KH, cfg.CW), dtype=ml_dtypes.float8_e4m3fn)
    for ci, c0 in enumerate(col_starts):
        blk = WqT[:, c0:c0 + cfg.CW]                       # [K, CW]
        # row k = kt*P + p -> [hc, p, kh, cw] with kt = hc*KH + kh
        blk = blk.reshape(2, KH, P, cfg.CW).transpose(0, 2, 1, 3)
        out[ci] = blk.astype(ml_dtypes.float8_e4m3fn)
    return out


def build_nc(cfg, sg, sd, so, nw_ones=True):
    """Build the single-core (SPMD) Bass program."""
    nc = bacc.Bacc()
    T, D, INTER, CW = cfg.T, cfg.D, cfg.INTER, cfg.CW
    TP, NP = cfg.TP, cfg.NP
    KT1, KT2, GCH, DCH = cfg.KT1, cfg.KT2, cfg.GCH, cfg.DCH
    QW = min(1024, D)             # quant sub-chunk width
    KTQ = QW // P                 # k-tiles per quant sub-chunk (psum slab)

    x_p = nc.declare_dram_parameter("x", [T, D], F32, isOutput=False)
    wg_p = nc.declare_dram_parameter("wg", [GCH, 2, P, KT1 // 2, CW], FP8, isOutput=False)
    wd_p = nc.declare_dram_parameter("wd", [DCH, 2, P, KT2 // 2, CW], FP8, isOutput=False)
    wo_p = nc.declare_dram_parameter("wo", [DCH, 2, P, KT1 // 2, CW], FP8, isOutput=False)
    if not nw_ones:
        nw_p = nc.declare_dram_parameter("nw", [1, D], F32, isOutput=False)
    out_p = nc.declare_dram_parameter("out", [T, D], F32, isOutput=True)

    c_gate = float(1.0 / (127.0 * sg))
    c_down = float(1.0 / (127.0 * sd))
    c_out = float(1.0 / (127.0 * so))

    with ExitStack() as ctx:
        tc = ctx.enter_context(tile.TileContext(nc))
        singles = ctx.enter_context(tc.tile_pool(name="singles", bufs=1))
        small = ctx.enter_context(tc.tile_pool(name="small", bufs=96))
        xin = ctx.enter_context(tc.tile_pool(name="xin", bufs=2))      # [P,QW] f32
        rts = ctx.enter_context(tc.tile_pool(name="rts", bufs=4))      # [P,QW] f32 scratch
        qt_pool = ctx.enter_context(tc.tile_pool(name="qt", bufs=2))   # [P,KT,TP*P] bf16
        wpool = ctx.enter_context(tc.tile_pool(name="wpool", bufs=8))  # [P,KT/2,CW] fp8
        gv = ctx.enter_context(tc.tile_pool(name="gv", bufs=2))        # [P,TP,CW] f32 per tag
        zpool = ctx.enter_context(tc.tile_pool(name="zpool", bufs=2))  # [P,TP,CW] f32
        mm_ps = ctx.enter_context(tc.tile_pool(name="mmps", bufs=6, space="PSUM"))
        tp_ps = ctx.enter_context(tc.tile_pool(name="tpps", bufs=2, space="PSUM"))
        dram = ctx.enter_context(tc.tile_pool(name="dram", bufs=1, space="DRAM"))

        eps_t = {}
        for ev in (1e-8, 1e-6):
            et = singles.tile([P, 1], F32, name=f"eps{ev}")
            nc.vector.memset(et, ev)
            eps_t[ev] = et
        ident = singles.tile([P, P], BF16, name="ident")
        make_identity(nc, ident)
        if not nw_ones:
            nw_bc = singles.tile([P, D], F32)
            nw_ap = nw_p[:]
            nc.sync.dma_start(
                out=nw_bc,
                in_=bass.AP(tensor=nw_ap.tensor, offset=nw_ap.offset, ap=[[0, P], [1, D]]),
            )

        def reduce_cols(parts, fn):
            """Combine [P,1] tiles with a binary DVE op; returns final tile."""
            while len(parts) > 1:
                nxt = []
                for i in range(0, len(parts) - 1, 2):
                    o = small.tile([P, 1], F32, tag="s", name="comb")
                    fn(o, parts[i], parts[i + 1])
                    nxt.append(o)
                if len(parts) % 2:
                    nxt.append(parts[-1])
                parts = nxt
            return parts[0]

        def quant_gen(src_ap, KTn, base_tt, qTt, c_t, c_const, eps, nw=False,
                      nw_eps=None, pe_tp=False):
            """Norm + int8-quant + transpose for TP token tiles of one piece.

            Quant scale is qs = 127/absmax(t2) (the rsqrt factor cancels
            algebraically between quant and dequant); the dequant scale
            c = (am*c_const)*r carries the norm factor r off the critical
            path. rt = (t2*qs + MAGIC) rounds to integer+MAGIC at the f32
            store; the -MAGIC subtract produces bf16 codes (exact for ints
            <= 127), then transposes move them into [K, token] layout.

            nw path: reference computes h1 = h*rsqrt(mean h^2 + nw_eps)*nw,
            then bit_linear renorms h2 = h1*rsqrt(mean h1^2 + eps); both
            fold into one per-token factor r = r1*r2 on (h*nw).  With
            nw == 1 the codes equal the plain path's and
            mean((h*r1)^2) == r1^2*ssq/DL, so the second elementwise stats
            pass collapses into [P,1] ops on ssq.
            """
            DL = KTn * P
            NQ = DL // QW
            for i in range(TP):
                tt = base_tt + i
                x_js = []
                for j in range(NQ):
                    x_j = xin.tile([P, QW], F32, tag="xin", bufs=10, name="xj")
                    # prologue: sync+scalar queues are otherwise idle, so
                    # split the x bolus across them; steady state: scalar
                    # queue (transposes own sync, writes own gpsimd)
                    if pe_tp:
                        eng = nc.sync if j % 2 == 1 else nc.scalar
                    else:
                        eng = nc.scalar
                    eng.dma_start(
                        out=x_j,
                        in_=src_ap[tt * P:(tt + 1) * P, j * QW:(j + 1) * QW])
                    x_js.append(x_j)
                sparts = []
                for j in range(NQ):
                    so_ = rts.tile([P, QW], F32, tag="sq", bufs=2)
                    ssj = small.tile([P, 1], F32, tag="s", name="ssj")
                    nc.scalar.activation(so_, x_js[j], AF.Square,
                                         accum_out=ssj)
                    sparts.append(ssj)
                ssq = reduce_cols(sparts, nc.vector.tensor_add)
                r = small.tile([P, 1], F32, tag="s")
                aparts = []
                if not nw:
                    std = small.tile([P, 1], F32, tag="s")
                    nc.scalar.activation(std, ssq, AF.Sqrt, scale=1.0 / DL, bias=eps_t[eps])
                    nc.vector.reciprocal(r, std)
                    for j in range(NQ):
                        amj = small.tile([P, 1], F32, tag="s", name="amj")
                        nc.vector.tensor_reduce(amj, x_js[j],
                                                axis=mybir.AxisListType.X, op=ALU.max,
                                                apply_absolute_value=True)
                        aparts.append(amj)
                else:
                    std1 = small.tile([P, 1], F32, tag="s")
                    nc.scalar.activation(std1, ssq, AF.Sqrt, scale=1.0 / DL,
                                         bias=eps_t[nw_eps])
                    r1 = small.tile([P, 1], F32, tag="s")
                    nc.vector.reciprocal(r1, std1)
                    if nw_ones:
                        ssq2 = ssq
                        for j in range(NQ):
                            amj = small.tile([P, 1], F32, tag="s", name="amj")
                            nc.vector.tensor_reduce(amj, x_js[j],
                                                    axis=mybir.AxisListType.X,
                                                    op=ALU.max,
                                                    apply_absolute_value=True)
                            aparts.append(amj)
                    else:
                        s2parts = []
                        for j in range(NQ):
                            t2j = rts.tile([P, QW], F32, tag="t2", bufs=1)
                            nc.vector.tensor_mul(t2j, x_js[j],
                                                 nw_bc[:, j * QW:(j + 1) * QW])
                            so2 = rts.tile([P, QW], F32, tag="sq", bufs=1)
                            ss2j = small.tile([P, 1], F32, tag="s", name="ss2j")
                            nc.scalar.activation(so2, t2j, AF.Square, accum_out=ss2j)
                            s2parts.append(ss2j)
                            amj = small.tile([P, 1], F32, tag="s", name="amj")
                            nc.vector.tensor_reduce(amj, t2j, axis=mybir.AxisListType.X,
                                                    op=ALU.max, apply_absolute_value=True)
                            aparts.append(amj)
                        ssq2 = reduce_cols(s2parts, nc.vector.tensor_add)
                    u = small.tile([P, 1], F32, tag="s")
                    nc.vector.tensor_mul(u, r1, r1)
                    w2 = small.tile([P, 1], F32, tag="s")
                    nc.vector.tensor_mul(w2, u, ssq2)
                    std2 = small.tile([P, 1], F32, tag="s")
                    nc.scalar.activation(std2, w2, AF.Sqrt, scale=1.0 / DL, bias=eps_t[eps])
                    r2 = small.tile([P, 1], F32, tag="s")
                    nc.vector.reciprocal(r2, std2)
                    nc.vector.tensor_mul(r, r1, r2)
                am = reduce_cols(aparts, nc.vector.tensor_max)
                invam = small.tile([P, 1], F32, tag="s")
                nc.vector.reciprocal(invam, am)
                qs = small.tile([P, 1], F32, tag="s")
                nc.vector.tensor_scalar_mul(qs, invam, 127.0)
                nc.vector.scalar_tensor_tensor(c_t[:, i:i + 1], am, c_const, r,
                                               op0=ALU.mult, op1=ALU.mult)
                # rt = t2*qs + MAGIC (f32 store rounds to nearest-even int)
                rtjs = []
                for j in range(NQ):
                    if not nw or nw_ones:
                        src_j = x_js[j]
                    else:
                        src_j = rts.tile([P, QW], F32, tag="t2", bufs=1)
                        nc.vector.tensor_mul(src_j, x_js[j],
                                             nw_bc[:, j * QW:(j + 1) * QW])
                    rt_j = rts.tile([P, QW], F32, tag="rt", bufs=4)
                    nc.vector.tensor_scalar(rt_j, src_j, scalar1=qs, scalar2=MAGIC,
                                            op0=ALU.mult, op1=ALU.add)
                    rtjs.append(rt_j)
                # subtract MAGIC -> bf16 codes, then transpose into qT.
                # Prologue piece (pe_tp): PE transposes batched 8-per-PSUM
                # bank + one DVE copy-out (short latency chain, no DGE).
                # Steady-state pieces: DMA-xbar transposes on the sync DGE
                # queue, which they get to themselves (the scalar-queue
                # xbar path corrupts data, so sync is the only valid
                # transpose queue).
                # qb stays on vector for BOTH parities: a scalar-engine qb
                # waits on the transpose that frees its buffer, and that
                # head-of-line blocks evac ACTIVATEs for 10s of us at piece
                # boundaries. bufs=8 reaches a full piece back so the
                # recycle wait is always long-satisfied.
                for j in range(NQ):
                    q_j = rts.tile([P, QW], BF16, tag="qb", bufs=8)
                    nc.vector.tensor_scalar_add(q_j, rtjs[j], -MAGIC)
                    dst = qTt[:, j * KTQ:(j + 1) * KTQ, i * P:(i + 1) * P]
                    # PE transposes only for the very first token tile: tile
                    # 1's PE transposes would sit in the PE FIFO ahead of the
                    # first product matmuls (which only need tile 0's codes)
                    # and idle the PE ~10us; tile 1 rides the sync xbar and
                    # overlaps the first matmul stream instead.
                    if pe_tp and i == 0:
                        tp = tp_ps.tile([P, KTQ, P], BF16, tag="tp")
                        for b in range(KTQ):
                            nc.tensor.transpose(
                                tp[:, b, :], q_j[:, b * P:(b + 1) * P], ident
                            )
                        nc.vector.tensor_copy(dst, tp)
                    else:
                        nc.sync.dma_start_transpose(dst, q_j)
                yield

        def mm_gen(w_p, nch, KTn, qTt, evac):
            KH = KTn // 2
            ii = list(range(TP))
            for c in range(nch):
                wts = []
                for g in range(2):
                    wt = wpool.tile([P, KH, CW], FP8, tag="w")
                    weng = nc.scalar if g == 0 else nc.gpsimd
                    weng.dma_start(out=wt, in_=w_p[c, g])
                    wts.append(wt)
                for i in ii:
                    ps = mm_ps.tile([P, CW], F32, tag="mm")
                    for kt in range(KTn):
                        rhs = wts[kt // KH][:, kt % KH, :]
                        nc.tensor.matmul(
                            ps,
                            lhsT=qTt[:, kt, i * P:(i + 1) * P],
                            rhs=rhs,
                            start=(kt == 0),
                            stop=(kt == KTn - 1),
                        )
                    evac.step(c, i, ps)  # psum bank freed per i-group
                evac.flush(c, ii)
                yield

        # ---- dram intermediates (per piece: no false cross-piece deps) ----
        z_ds = [dram.tile([T // NP, INTER], F32, name=f"z{p}", tag=f"z{p}")
                for p in range(NP)]
        z_rs = [zd[:].rearrange("(a p) n -> p a n", p=P) for zd in z_ds]
        h_ds = [dram.tile([T // NP, D], F32, name=f"h{p}", tag=f"h{p}")
                for p in range(NP)]
        h_rs = [hd[:].rearrange("(a p) n -> p a n", p=P) for hd in h_ds]
        out_r = out_p[:].rearrange("(a p) n -> p a n", p=P)
        st = {}

        class Evac1:
            # chunk order v0,g0,v1,g1,...: v dequantized x c1^2 (extra c1
            # pre-applies gate dequant); z = (psum_g * sigmoid(psum_g*c1)) * v
            def __init__(self, c1h, piece):
                self.c1h = c1h
                self.piece = piece

            def step(self, c, i, ps):
                cc = self.c1h[:, i:i + 1]
                if c % 2 == 0:
                    if st.get("v_new", True):
                        st["v"] = gv.tile([P, TP, CW], F32, tag="v", name="v_t")
                        st["v_new"] = False
                    nc.vector.tensor_scalar(st["v"][:, i, :], ps, scalar1=cc,
                                            scalar2=cc, op0=ALU.mult, op1=ALU.mult)
                else:
                    if st.get("sig_new", True):
                        st["sig"] = gv.tile([P, TP, CW], F32, tag="sig", bufs=1, name="sig_t")
                        st["z"] = zpool.tile([P, TP, CW], F32, tag="z", name="z_t")
                        st["sig_new"] = False
                    nc.scalar.activation(st["sig"][:, i, :], ps, AF.Sigmoid, scale=cc)
                    nc.vector.tensor_mul(st["z"][:, i, :], ps, st["sig"][:, i, :])

            def flush(self, c, ii):
                if c % 2 == 0:
                    st["sig_new"] = True
                else:
                    st["v_new"] = True
                    lo, hi = ii[0], ii[-1] + 1
                    z_t = st["z"]
                    nc.vector.tensor_mul(z_t[:, lo:hi, :], z_t[:, lo:hi, :],
                                         st["v"][:, lo:hi, :])
                    nc.gpsimd.dma_start(
                        out=z_rs[self.piece][:, lo:hi, (c // 2) * CW:(c // 2 + 1) * CW],
                        in_=z_t[:, lo:hi, :]
                    )

        class EvacPlain:
            def __init__(self, c_th, dst_r, row0, final=False):
                self.c_th = c_th
                self.dst_r = dst_r
                self.row0 = row0
                self.final = final

            def step(self, c, i, ps):
                if st.get("o_new", True):
                    st["o"] = zpool.tile([P, TP, CW], F32, tag="z", name="o_t")
                    st["o_new"] = False
                cc = self.c_th[:, i:i + 1]
                if i % 2 == 0:
                    nc.vector.tensor_scalar(st["o"][:, i, :], ps, scalar1=cc,
                                            scalar2=None, op0=ALU.mult)
                else:
                    nc.scalar.activation(st["o"][:, i, :], ps, AF.Copy, scale=cc)

            def flush(self, c, ii):
                st["o_new"] = True
                if self.final:
                    # last piece: per-i writes on alternating queues so the
                    # post-last-matmul DMA drain is ~1 write, not a backlog
                    for i in ii:
                        eng = nc.gpsimd if i % 2 == 0 else nc.sync
                        eng.dma_start(
                            out=self.dst_r[:, self.row0 + i:self.row0 + i + 1,
                                           c * CW:(c + 1) * CW],
                            in_=st["o"][:, i:i + 1, :]
                        )
                    return
                lo, hi = ii[0], ii[-1] + 1
                row0 = self.row0 + lo
                nc.gpsimd.dma_start(
                    out=self.dst_r[:, row0:row0 + hi - lo, c * CW:(c + 1) * CW],
                    in_=st["o"][:, lo:hi, :]
                )

        # ---- pipelined stages: emission INTERLEAVED so quant(stage k+1)
        # overlaps mm(stage k) on every engine's instruction stream ----
        def stage_factory(L, p):
            def mk():
                ct = singles.tile([P, TP], F32, name=f"c{L}_{p}")
                if L == 1:
                    qT = qt_pool.tile([P, KT1, TP * P], BF16, tag="qt")
                    qg = quant_gen(x_p[:], KT1, p * TP, qT, ct, c_gate, 1e-8,
                                   pe_tp=(p == 0))
                    mmf = lambda: mm_gen(wg_p, GCH, KT1, qT, Evac1(ct, p))
                    return qg, mmf
                if L == 2:
                    qT = qt_pool.tile([P, KT2, TP * P], BF16, tag="qt")
                    qg = quant_gen(z_ds[p][:], KT2, 0, qT, ct, c_down, 1e-8)
                    mmf = lambda: mm_gen(wd_p, DCH, KT2, qT,
                                         EvacPlain(ct, h_rs[p], 0))
                    return qg, mmf
                qT = qt_pool.tile([P, KT1, TP * P], BF16, tag="qt")
                qg = quant_gen(h_ds[p][:], KT1, 0, qT, ct, c_out, 1e-8,
                               nw=True, nw_eps=1e-6)
                mmf = lambda: mm_gen(wo_p, DCH, KT1, qT,
                                     EvacPlain(ct, out_r, p * TP,
                                               final=(p == NP - 1)))
                return qg, mmf
            return mk

        stage_mks = [stage_factory(L, p) for L in (1, 2, 3) for p in range(NP)]
        qg0, mmf = stage_mks[0]()
        for _ in qg0:
            pass
        for k in range(len(stage_mks)):
            mm = mmf()
            if k + 1 < len(stage_mks):
                qn, mmf = stage_mks[k + 1]()
            else:
                qn = None
            # Emit the ENTIRE next-stage quant right after the first chunk:
            # gradual interleave leaves the quant tail sandwiched behind
            # psum-gated evac ops at the end of each engine FIFO, which
            # re-creates a ~13us late-quant equilibrium at every piece
            # boundary. Early full emission lets it run at piece start.
            # (A reader emitted before its writer does NOT get a dependency
            # — emission order defines the DAG — so the quant must be fully
            # emitted before any matmul that reads its codes.)
            ci = 0
            for _ in mm:
                ci += 1
                if qn is not None and ci >= 1:
                    for _ in qn:
                        pass
                    qn = None
    return nc


def prepare_inputs(condition, w_gate, w_down, norm_weight, w_out, cfg, n_cores=N_CORES):
    """Host-side: quantize+pack weights, shard tokens. Returns (in_maps, scales)."""
    TOK = condition.shape[0] * condition.shape[1]
    X = np.ascontiguousarray(condition.reshape(TOK, cfg.D).astype(np.float32, copy=False))

    Wg, sg = host_weight_quant(np.asarray(w_gate, dtype=np.float32))
    Wd, sd = host_weight_quant(np.asarray(w_down, dtype=np.float32))
    Wo, so = host_weight_quant(np.asarray(w_out, dtype=np.float32))

    # L1 chunk order interleaves v/gate so swiglu can fuse per chunk pair
    l1_cols = []
    for i in range(cfg.INTER // cfg.CW):
        l1_cols += [cfg.INTER + i * cfg.CW, i * cfg.CW]
    WG = pack_weight(Wg.T, l1_cols, cfg)
    WD = pack_weight(Wd.T, [i * cfg.CW for i in range(cfg.D // cfg.CW)], cfg)
    WO = pack_weight(Wo.T, [i * cfg.CW for i in range(cfg.D // cfg.CW)], cfg)

    nw = np.ascontiguousarray(np.asarray(norm_weight, dtype=np.float32).reshape(1, cfg.D))
    nw_ones = bool(np.all(nw == np.float32(1.0)))

    in_maps = []
    for i in range(n_cores):
        m = {
            "x": np.ascontiguousarray(X[i * cfg.T:(i + 1) * cfg.T]),
            "wg": WG, "wd": WD, "wo": WO,
        }
        if not nw_ones:
            m["nw"] = nw
        in_maps.append(m)
    return in_maps, (sg, sd, so), nw_ones


def run(condition, w_gate, w_down, norm_weight, w_out, cfg=None, trace=False, tmpdir=None):
    from concourse.bass_utils import run_bass_kernel_spmd
    if cfg is None:
        cfg = Cfg()
    in_maps, (sg, sd, so), nw_ones = prepare_inputs(condition, w_gate, w_down,
                                                     norm_weight, w_out, cfg)
    nc = build_nc(cfg, sg, sd, so, nw_ones=nw_ones)
    nc.finalize()
    # transient NRT_EXEC_UNIT_UNRECOVERABLE device crashes recover on retry
    last_err = None
    for attempt in range(3):
        try:
            res = run_bass_kernel_spmd(nc, in_maps, list(range(N_CORES)), trace=trace,
                                       tmpdir=tmpdir)
            break
        except Exception as e:  # noqa: BLE001
            last_err = e
            if attempt == 2:
                raise
            import time as _time
            _time.sleep(20)
    outs = np.concatenate([np.asarray(res.results[i]["out"]) for i in range(N_CORES)], axis=0)
    B, S = condition.shape[0], condition.shape[1]
    Pfull = outs.reshape(B, S, cfg.D)
    H = cfg.D // 2
    return (Pfull[..., :H], Pfull[..., H:]), res


def kernel(condition, w_gate, w_down, norm_weight, w_out):
    (scale, shift), _ = run(condition, w_gate, w_down, norm_weight, w_out)
    return scale, shift


# revision 69
# speedup vs baseline: 1.2000x; 1.0052x over previous
"""Trainium2 Bass kernel for nn_AdaLNConditioning (HGRNBitMLP + AdaLN head).

Strategy:
- Data-parallel over tokens: 8192 tokens -> 1024 per core, no collectives.
- Host precomputes ternary weight quantization (BitNet b1.58 global-mean
  scale) and packs transposed weight tiles in streaming order as fp8e4
  (ternary {-1,0,1} is exact in e4m3; PE mixed bf16 x fp8 matmul verified
  bit-exact on HW). Halves weight HBM traffic and DGE descriptor count
  vs bf16.
- On device, per token tile [128, D]: RMSNorm stats + per-token int8
  quantization (round-to-nearest-even via the 1.5*2^23 magic constant,
  bit-exact with jnp.round), quantized codes stored as bf16 (integers
  <= 127 are exact in bf16), transposed into [K, token] layout with PE
  transposes batched 8-per-PSUM-bank + one DVE copy-out in the prologue
  (no DGE dependency) and DMA-xbar transposes on the sync queue in
  steady state.
- Matmuls run on bf16 codes x fp8 weights with f32 PSUM accumulation ->
  exact integer arithmetic; per-token dequant scale applied at PSUM
  evacuation (fused into ScalarE/VectorE copy). The bf16 N=512 matmul
  stream is the PE roofline for this problem: fp8 DoubleRow fails
  accuracy (e4m3 activations ~2.8%/layer vs the 2e-2 budget; an exact
  hi/lo split costs 2 DoubleRow matmuls = 1.39x bf16), and uint8 matmul
  is rejected by walrus codegen (s3d3_mm_dtype ISA check).
- Each layer is processed in NP=4 pieces of TP=2 token tiles (rather
  than 2 halves of 4): the first piece's quant is the only exposed
  (PE-idle) latency, so halving the piece cuts the prologue roughly in
  half. Weights stream once per piece (2x the traffic of the half
  schedule) which still fits the DMA budget when split across queues.
- swiglu intermediate z and down-proj output h round-trip through HBM
  in f32 (bf16 storage costs ~1.5e-2 rel err; f32 keeps e2e ~1.4e-3).
- DMA queues (only SP/Activation/gpsimd can initiate DMAs): sync DGE =
  code transposes only (the scalar-queue xbar path corrupts data, so
  sync is the only valid transpose queue); scalar DGE = weight
  half-chunk 0 + activation reads (x/z/h); gpsimd DGE = weight
  half-chunk 1 + z/h/out writes. Prologue piece 0 splits its x loads
  across sync+scalar (both otherwise idle there).
- norm_weight is all-ones for this module (checked on host): the AdaLN
  RMSNorm folds into the L3 bit_linear renorm as a pure [P,1] scalar
  chain on the existing ssq stats; no nw broadcast or second stats
  pass. A general nw path is kept as fallback.
"""

import sys
from contextlib import ExitStack

import numpy as np
import ml_dtypes

sys.path.insert(0, "/opt/trn_rl_repo")

import concourse.bass as bass  # noqa: E402
import concourse.tile as tile  # noqa: E402
from concourse import bacc  # noqa: E402
from concourse import mybir  # noqa: E402
from concourse.masks import make_identity  # noqa: E402

AF = mybir.ActivationFunctionType
ALU = mybir.AluOpType
F32 = mybir.dt.float32
BF16 = mybir.dt.bfloat16
FP8 = mybir.dt.float8e4

P = 128
MAGIC = 12582912.0  # 1.5 * 2**23: add+store rounds f32 to nearest-even integer
N_CORES = 8


class Cfg:
    def __init__(self, T=1024, D=4096, INTER=4096, CW=512, TP=2):
        self.T = T            # tokens per core
        self.D = D            # model dim (k of L1/L3, out of L2/L3)
        self.INTER = INTER    # swiglu intermediate
        self.CW = CW          # output-chunk width (matmul moving free dim)
        self.TT = T // P      # token tiles per core
        self.TP = TP          # token tiles per piece
        self.NP = self.TT // TP
        self.GCH = 2 * INTER // CW  # L1 chunks (v/gate interleaved)
        self.DCH = D // CW          # L2/L3 chunks
        self.KT1 = D // P
        self.KT2 = INTER // P


def host_weight_quant(w):
    """BitNet ternary quant. Returns (codes {-1,0,1} f32, scale) matching
    jnp: scale = 1/clip(mean|w|, 1e-5); q = clip(round(w*scale), -1, 1)."""
    mean_abs = np.mean(np.abs(w), dtype=np.float64).astype(np.float32)
    s = np.float32(1.0) / np.maximum(mean_abs, np.float32(1e-5))
    q = np.clip(np.round(w * s), -1, 1).astype(np.float32)
    return q, s


def pack_weight(WqT, col_starts, cfg):
    """Pack WqT [K, O] into [n_chunks, 2, P, KT/2, CW] fp8 half-chunk
    streaming layout: per chunk, two DMA-able halves, each with 8KB of
    contiguous per-partition data (k-tile-major within the half)."""
    K = WqT.shape[0]
    KT = K // P
    KH = KT // 2
    out = np.empty((len(col_starts), 2, P, KH, cfg.CW), dtype=ml_dtypes.float8_e4m3fn)
    for ci, c0 in enumerate(col_starts):
        blk = WqT[:, c0:c0 + cfg.CW]                       # [K, CW]
        # row k = kt*P + p -> [hc, p, kh, cw] with kt = hc*KH + kh
        blk = blk.reshape(2, KH, P, cfg.CW).transpose(0, 2, 1, 3)
        out[ci] = blk.astype(ml_dtypes.float8_e4m3fn)
    return out


def build_nc(cfg, sg, sd, so, nw_ones=True):
    """Build the single-core (SPMD) Bass program."""
    nc = bacc.Bacc()
    T, D, INTER, CW = cfg.T, cfg.D, cfg.INTER, cfg.CW
    TP, NP = cfg.TP, cfg.NP
    KT1, KT2, GCH, DCH = cfg.KT1, cfg.KT2, cfg.GCH, cfg.DCH
    QW = min(1024, D)             # quant sub-chunk width
    KTQ = QW // P                 # k-tiles per quant sub-chunk (psum slab)

    x_p = nc.declare_dram_parameter("x", [T, D], F32, isOutput=False)
    wg_p = nc.declare_dram_parameter("wg", [GCH, 2, P, KT1 // 2, CW], FP8, isOutput=False)
    wd_p = nc.declare_dram_parameter("wd", [DCH, 2, P, KT2 // 2, CW], FP8, isOutput=False)
    wo_p = nc.declare_dram_parameter("wo", [DCH, 2, P, KT1 // 2, CW], FP8, isOutput=False)
    if not nw_ones:
        nw_p = nc.declare_dram_parameter("nw", [1, D], F32, isOutput=False)
    out_p = nc.declare_dram_parameter("out", [T, D], F32, isOutput=True)

    c_gate = float(1.0 / (127.0 * sg))
    c_down = float(1.0 / (127.0 * sd))
    c_out = float(1.0 / (127.0 * so))

    with ExitStack() as ctx:
        tc = ctx.enter_context(tile.TileContext(nc))
        singles = ctx.enter_context(tc.tile_pool(name="singles", bufs=1))
        small = ctx.enter_context(tc.tile_pool(name="small", bufs=96))
        xin = ctx.enter_context(tc.tile_pool(name="xin", bufs=2))      # [P,QW] f32
        rts = ctx.enter_context(tc.tile_pool(name="rts", bufs=4))      # [P,QW] f32 scratch
        qt_pool = ctx.enter_context(tc.tile_pool(name="qt", bufs=2))   # [P,KT,TP*P] bf16
        wpool = ctx.enter_context(tc.tile_pool(name="wpool", bufs=8))  # [P,KT/2,CW] fp8
        gv = ctx.enter_context(tc.tile_pool(name="gv", bufs=2))        # [P,TP,CW] f32 per tag
        zpool = ctx.enter_context(tc.tile_pool(name="zpool", bufs=2))  # [P,TP,CW] f32
        mm_ps = ctx.enter_context(tc.tile_pool(name="mmps", bufs=6, space="PSUM"))
        tp_ps = ctx.enter_context(tc.tile_pool(name="tpps", bufs=2, space="PSUM"))
        dram = ctx.enter_context(tc.tile_pool(name="dram", bufs=1, space="DRAM"))

        eps_t = {}
        for ev in (1e-8, 1e-6):
            et = singles.tile([P, 1], F32, name=f"eps{ev}")
            nc.vector.memset(et, ev)
            eps_t[ev] = et
        ident = singles.tile([P, P], BF16, name="ident")
        make_identity(nc, ident)
        if not nw_ones:
            nw_bc = singles.tile([P, D], F32)
            nw_ap = nw_p[:]
            nc.sync.dma_start(
                out=nw_bc,
                in_=bass.AP(tensor=nw_ap.tensor, offset=nw_ap.offset, ap=[[0, P], [1, D]]),
            )

        def reduce_cols(parts, fn):
            """Combine [P,1] tiles with a binary DVE op; returns final tile."""
            while len(parts) > 1:
                nxt = []
                for i in range(0, len(parts) - 1, 2):
                    o = small.tile([P, 1], F32, tag="s", name="comb")
                    fn(o, parts[i], parts[i + 1])
                    nxt.append(o)
                if len(parts) % 2:
                    nxt.append(parts[-1])
                parts = nxt
            return parts[0]

        def quant_gen(src_ap, KTn, base_tt, qTt, c_t, c_const, eps, nw=False,
                      nw_eps=None, pe_tp=False):
            """Norm + int8-quant + transpose for TP token tiles of one piece.

            Quant scale is qs = 127/absmax(t2) (the rsqrt factor cancels
            algebraically between quant and dequant); the dequant scale
            c = (am*c_const)*r carries the norm factor r off the critical
            path. rt = (t2*qs + MAGIC) rounds to integer+MAGIC at the f32
            store; the -MAGIC subtract produces bf16 codes (exact for ints
            <= 127), then transposes move them into [K, token] layout.

            nw path: reference computes h1 = h*rsqrt(mean h^2 + nw_eps)*nw,
            then bit_linear renorms h2 = h1*rsqrt(mean h1^2 + eps); both
            fold into one per-token factor r = r1*r2 on (h*nw).  With
            nw == 1 the codes equal the plain path's and
            mean((h*r1)^2) == r1^2*ssq/DL, so the second elementwise stats
            pass collapses into [P,1] ops on ssq.
            """
            DL = KTn * P
            NQ = DL // QW
            for i in range(TP):
                tt = base_tt + i
                x_js = []
                for j in range(NQ):
                    x_j = xin.tile([P, QW], F32, tag="xin", bufs=10, name="xj")
                    # prologue: sync+scalar queues are otherwise idle, so
                    # split the x bolus across them; steady state: scalar
                    # queue (transposes own sync, writes own gpsimd)
                    if pe_tp:
                        eng = nc.sync if j % 2 == 1 else nc.scalar
                    else:
                        eng = nc.scalar
                    eng.dma_start(
                        out=x_j,
                        in_=src_ap[tt * P:(tt + 1) * P, j * QW:(j + 1) * QW])
                    x_js.append(x_j)
                sparts = []
                for j in range(NQ):
                    so_ = rts.tile([P, QW], F32, tag="sq", bufs=2)
                    ssj = small.tile([P, 1], F32, tag="s", name="ssj")
                    nc.scalar.activation(so_, x_js[j], AF.Square,
                                         accum_out=ssj)
                    sparts.append(ssj)
                ssq = reduce_cols(sparts, nc.vector.tensor_add)
                r = small.tile([P, 1], F32, tag="s")
                aparts = []
                if not nw:
                    std = small.tile([P, 1], F32, tag="s")
                    nc.scalar.activation(std, ssq, AF.Sqrt, scale=1.0 / DL, bias=eps_t[eps])
                    nc.vector.reciprocal(r, std)
                    for j in range(NQ):
                        amj = small.tile([P, 1], F32, tag="s", name="amj")
                        nc.vector.tensor_reduce(amj, x_js[j],
                                                axis=mybir.AxisListType.X, op=ALU.max,
                                                apply_absolute_value=True)
                        aparts.append(amj)
                else:
                    std1 = small.tile([P, 1], F32, tag="s")
                    nc.scalar.activation(std1, ssq, AF.Sqrt, scale=1.0 / DL,
                                         bias=eps_t[nw_eps])
                    r1 = small.tile([P, 1], F32, tag="s")
                    nc.vector.reciprocal(r1, std1)
                    if nw_ones:
                        ssq2 = ssq
                        for j in range(NQ):
                            amj = small.tile([P, 1], F32, tag="s", name="amj")
                            nc.vector.tensor_reduce(amj, x_js[j],
                                                    axis=mybir.AxisListType.X,
                                                    op=ALU.max,
                                                    apply_absolute_value=True)
                            aparts.append(amj)
                    else:
                        s2parts = []
                        for j in range(NQ):
                            t2j = rts.tile([P, QW], F32, tag="t2", bufs=1)
                            nc.vector.tensor_mul(t2j, x_js[j],
                                                 nw_bc[:, j * QW:(j + 1) * QW])
                            so2 = rts.tile([P, QW], F32, tag="sq", bufs=1)
                            ss2j = small.tile([P, 1], F32, tag="s", name="ss2j")
                            nc.scalar.activation(so2, t2j, AF.Square, accum_out=ss2j)
                            s2parts.append(ss2j)
                            amj = small.tile([P, 1], F32, tag="s", name="amj")
                            nc.vector.tensor_reduce(amj, t2j, axis=mybir.AxisListType.X,
                                                    op=ALU.max, apply_absolute_value=True)
                            aparts.append(amj)
                        ssq2 = reduce_cols(s2parts, nc.vector.tensor_add)
                    u = small.tile([P, 1], F32, tag="s")
                    nc.vector.tensor_mul(u, r1, r1)
                    w2 = small.tile([P, 1], F32, tag="s")
                    nc.vector.tensor_mul(w2, u, ssq2)
                    std2 = small.tile([P, 1], F32, tag="s")
                    nc.scalar.activation(std2, w2, AF.Sqrt, scale=1.0 / DL, bias=eps_t[eps])
                    r2 = small.tile([P, 1], F32, tag="s")
                    nc.vector.reciprocal(r2, std2)
                    nc.vector.tensor_mul(r, r1, r2)
                am = reduce_cols(aparts, nc.vector.tensor_max)
                invam = small.tile([P, 1], F32, tag="s")
                nc.vector.reciprocal(invam, am)
                qs = small.tile([P, 1], F32, tag="s")
                nc.vector.tensor_scalar_mul(qs, invam, 127.0)
                nc.vector.scalar_tensor_tensor(c_t[:, i:i + 1], am, c_const, r,
                                               op0=ALU.mult, op1=ALU.mult)
                # rt = t2*qs + MAGIC (f32 store rounds to nearest-even int)
                # rt = t2*qs + MAGIC, then qb = rt - MAGIC -> bf16 codes,
                # then transpose into qT. rt/qb are emitted interleaved per
                # sub-chunk so the first transpose batch starts after qb(j0)
                # rather than after all four rt's.
                # Prologue piece (pe_tp): PE transposes batched 8-per-PSUM
                # bank + one DVE copy-out (short latency chain, no DGE).
                # Steady-state pieces: DMA-xbar transposes on the sync DGE
                # queue, which they get to themselves (the scalar-queue
                # xbar path corrupts data, so sync is the only valid
                # transpose queue).
                # Steady-state qb stays on vector: a scalar-engine qb waits
                # on the transpose that frees its buffer, head-of-line
                # blocking evac ACTIVATEs at piece boundaries (bufs=8
                # reaches a full piece back). Tile 0 of the prologue piece
                # runs qb on ScalarE with a DEDICATED ring (mixed writers
                # on one ring race) to shorten the DVE critical path to the
                # first matmuls.
                for j in range(NQ):
                    if not nw or nw_ones:
                        src_j = x_js[j]
                    else:
                        src_j = rts.tile([P, QW], F32, tag="t2", bufs=1)
                        nc.vector.tensor_mul(src_j, x_js[j],
                                             nw_bc[:, j * QW:(j + 1) * QW])
                    rt_j = rts.tile([P, QW], F32, tag="rt", bufs=4)
                    nc.vector.tensor_scalar(rt_j, src_j, scalar1=qs, scalar2=MAGIC,
                                            op0=ALU.mult, op1=ALU.add)
                    if pe_tp and i == 0:
                        q_j = rts.tile([P, QW], BF16, tag="qb0", bufs=4)
                        nc.scalar.activation(q_j, rt_j, AF.Copy, bias=-MAGIC)
                    else:
                        q_j = rts.tile([P, QW], BF16, tag="qb", bufs=8)
                        nc.vector.tensor_scalar_add(q_j, rt_j, -MAGIC)
                    dst = qTt[:, j * KTQ:(j + 1) * KTQ, i * P:(i + 1) * P]
                    # PE transposes only for the very first token tile: tile
                    # 1's PE transposes would sit in the PE FIFO ahead of the
                    # first product matmuls (which only need tile 0's codes)
                    # and idle the PE ~10us; tile 1 rides the sync xbar and
                    # overlaps the first matmul stream instead.
                    if pe_tp and i == 0:
                        tp = tp_ps.tile([P, KTQ, P], BF16, tag="tp")
                        for b in range(KTQ):
                            nc.tensor.transpose(
                                tp[:, b, :], q_j[:, b * P:(b + 1) * P], ident
                            )
                        nc.vector.tensor_copy(dst, tp)
                    else:
                        nc.sync.dma_start_transpose(dst, q_j)
                yield

        def mm_gen(w_p, nch, KTn, qTt, evac):
            KH = KTn // 2
            ii = list(range(TP))
            for c in range(nch):
                wts = []
                for g in range(2):
                    wt = wpool.tile([P, KH, CW], FP8, tag="w")
                    weng = nc.scalar if g == 0 else nc.gpsimd
                    weng.dma_start(out=wt, in_=w_p[c, g])
                    wts.append(wt)
                for i in ii:
                    ps = mm_ps.tile([P, CW], F32, tag="mm")
                    for kt in range(KTn):
                        rhs = wts[kt // KH][:, kt % KH, :]
                        nc.tensor.matmul(
                            ps,
                            lhsT=qTt[:, kt, i * P:(i + 1) * P],
                            rhs=rhs,
                            start=(kt == 0),
                            stop=(kt == KTn - 1),
                        )
                    evac.step(c, i, ps)  # psum bank freed per i-group
                evac.flush(c, ii)
                yield

        # ---- dram intermediates (per piece: no false cross-piece deps) ----
        z_ds = [dram.tile([T // NP, INTER], F32, name=f"z{p}", tag=f"z{p}")
                for p in range(NP)]
        z_rs = [zd[:].rearrange("(a p) n -> p a n", p=P) for zd in z_ds]
        h_ds = [dram.tile([T // NP, D], F32, name=f"h{p}", tag=f"h{p}")
                for p in range(NP)]
        h_rs = [hd[:].rearrange("(a p) n -> p a n", p=P) for hd in h_ds]
        out_r = out_p[:].rearrange("(a p) n -> p a n", p=P)
        st = {}

        class Evac1:
            # chunk order v0,g0,v1,g1,...: v dequantized x c1^2 (extra c1
            # pre-applies gate dequant); z = (psum_g * sigmoid(psum_g*c1)) * v
            def __init__(self, c1h, piece):
                self.c1h = c1h
                self.piece = piece

            def step(self, c, i, ps):
                cc = self.c1h[:, i:i + 1]
                if c % 2 == 0:
                    if st.get("v_new", True):
                        st["v"] = gv.tile([P, TP, CW], F32, tag="v", name="v_t")
                        st["v_new"] = False
                    nc.vector.tensor_scalar(st["v"][:, i, :], ps, scalar1=cc,
                                            scalar2=cc, op0=ALU.mult, op1=ALU.mult)
                else:
                    if st.get("sig_new", True):
                        st["sig"] = gv.tile([P, TP, CW], F32, tag="sig", bufs=1, name="sig_t")
                        st["z"] = zpool.tile([P, TP, CW], F32, tag="z", name="z_t")
                        st["sig_new"] = False
                    nc.scalar.activation(st["sig"][:, i, :], ps, AF.Sigmoid, scale=cc)
                    nc.vector.tensor_mul(st["z"][:, i, :], ps, st["sig"][:, i, :])

            def flush(self, c, ii):
                if c % 2 == 0:
                    st["sig_new"] = True
                else:
                    st["v_new"] = True
                    lo, hi = ii[0], ii[-1] + 1
                    z_t = st["z"]
                    nc.vector.tensor_mul(z_t[:, lo:hi, :], z_t[:, lo:hi, :],
                                         st["v"][:, lo:hi, :])
                    nc.gpsimd.dma_start(
                        out=z_rs[self.piece][:, lo:hi, (c // 2) * CW:(c // 2 + 1) * CW],
                        in_=z_t[:, lo:hi, :]
                    )

        class EvacPlain:
            def __init__(self, c_th, dst_r, row0, final=False):
                self.c_th = c_th
                self.dst_r = dst_r
                self.row0 = row0
                self.final = final

            def step(self, c, i, ps):
                if st.get("o_new", True):
                    st["o"] = zpool.tile([P, TP, CW], F32, tag="z", name="o_t")
                    st["o_new"] = False
                cc = self.c_th[:, i:i + 1]
                if i % 2 == 0:
                    nc.vector.tensor_scalar(st["o"][:, i, :], ps, scalar1=cc,
                                            scalar2=None, op0=ALU.mult)
                else:
                    nc.scalar.activation(st["o"][:, i, :], ps, AF.Copy, scale=cc)

            def flush(self, c, ii):
                st["o_new"] = True
                if self.final:
                    # last piece: per-i writes on alternating queues so the
                    # post-last-matmul DMA drain is ~1 write, not a backlog
                    for i in ii:
                        eng = nc.gpsimd if i % 2 == 0 else nc.sync
                        eng.dma_start(
                            out=self.dst_r[:, self.row0 + i:self.row0 + i + 1,
                                           c * CW:(c + 1) * CW],
                            in_=st["o"][:, i:i + 1, :]
                        )
                    return
                lo, hi = ii[0], ii[-1] + 1
                row0 = self.row0 + lo
                nc.gpsimd.dma_start(
                    out=self.dst_r[:, row0:row0 + hi - lo, c * CW:(c + 1) * CW],
                    in_=st["o"][:, lo:hi, :]
                )

        # ---- pipelined stages: emission INTERLEAVED so quant(stage k+1)
        # overlaps mm(stage k) on every engine's instruction stream ----
        def stage_factory(L, p):
            def mk():
                ct = singles.tile([P, TP], F32, name=f"c{L}_{p}")
                if L == 1:
                    qT = qt_pool.tile([P, KT1, TP * P], BF16, tag="qt")
                    qg = quant_gen(x_p[:], KT1, p * TP, qT, ct, c_gate, 1e-8,
                                   pe_tp=(p == 0))
                    mmf = lambda: mm_gen(wg_p, GCH, KT1, qT, Evac1(ct, p))
                    return qg, mmf
                if L == 2:
                    qT = qt_pool.tile([P, KT2, TP * P], BF16, tag="qt")
                    qg = quant_gen(z_ds[p][:], KT2, 0, qT, ct, c_down, 1e-8)
                    mmf = lambda: mm_gen(wd_p, DCH, KT2, qT,
                                         EvacPlain(ct, h_rs[p], 0))
                    return qg, mmf
                qT = qt_pool.tile([P, KT1, TP * P], BF16, tag="qt")
                qg = quant_gen(h_ds[p][:], KT1, 0, qT, ct, c_out, 1e-8,
                               nw=True, nw_eps=1e-6)
                mmf = lambda: mm_gen(wo_p, DCH, KT1, qT,
                                     EvacPlain(ct, out_r, p * TP,
                                               final=(p == NP - 1)))
                return qg, mmf
            return mk

        stage_mks = [stage_factory(L, p) for L in (1, 2, 3) for p in range(NP)]
        qg0, mmf = stage_mks[0]()
        for _ in qg0:
            pass
        for k in range(len(stage_mks)):
            mm = mmf()
            if k + 1 < len(stage_mks):
                qn, mmf = stage_mks[k + 1]()
            else:
                qn = None
            # Emit the ENTIRE next-stage quant right after the first chunk:
            # gradual interleave leaves the quant tail sandwiched behind
            # psum-gated evac ops at the end of each engine FIFO, which
            # re-creates a ~13us late-quant equilibrium at every piece
            # boundary. Early full emission lets it run at piece start.
            # (A reader emitted before its writer does NOT get a dependency
            # — emission order defines the DAG — so the quant must be fully
            # emitted before any matmul that reads its codes.)
            ci = 0
            for _ in mm:
                ci += 1
                if qn is not None and ci >= 1:
                    for _ in qn:
                        pass
                    qn = None
    return nc


def prepare_inputs(condition, w_gate, w_down, norm_weight, w_out, cfg, n_cores=N_CORES):
    """Host-side: quantize+pack weights, shard tokens. Returns (in_maps, scales)."""
    TOK = condition.shape[0] * condition.shape[1]
    X = np.ascontiguousarray(condition.reshape(TOK, cfg.D).astype(np.float32, copy=False))

    Wg, sg = host_weight_quant(np.asarray(w_gate, dtype=np.float32))
    Wd, sd = host_weight_quant(np.asarray(w_down, dtype=np.float32))
    Wo, so = host_weight_quant(np.asarray(w_out, dtype=np.float32))

    # L1 chunk order interleaves v/gate so swiglu can fuse per chunk pair
    l1_cols = []
    for i in range(cfg.INTER // cfg.CW):
        l1_cols += [cfg.INTER + i * cfg.CW, i * cfg.CW]
    WG = pack_weight(Wg.T, l1_cols, cfg)
    WD = pack_weight(Wd.T, [i * cfg.CW for i in range(cfg.D // cfg.CW)], cfg)
    WO = pack_weight(Wo.T, [i * cfg.CW for i in range(cfg.D // cfg.CW)], cfg)

    nw = np.ascontiguousarray(np.asarray(norm_weight, dtype=np.float32).reshape(1, cfg.D))
    nw_ones = bool(np.all(nw == np.float32(1.0)))

    in_maps = []
    for i in range(n_cores):
        m = {
            "x": np.ascontiguousarray(X[i * cfg.T:(i + 1) * cfg.T]),
            "wg": WG, "wd": WD, "wo": WO,
        }
        if not nw_ones:
            m["nw"] = nw
        in_maps.append(m)
    return in_maps, (sg, sd, so), nw_ones


def run(condition, w_gate, w_down, norm_weight, w_out, cfg=None, trace=False, tmpdir=None):
    from concourse.bass_utils import run_bass_kernel_spmd
    if cfg is None:
        cfg = Cfg()
    in_maps, (sg, sd, so), nw_ones = prepare_inputs(condition, w_gate, w_down,
                                                     norm_weight, w_out, cfg)
    nc = build_nc(cfg, sg, sd, so, nw_ones=nw_ones)
    nc.finalize()
    # transient NRT_EXEC_UNIT_UNRECOVERABLE device crashes recover on retry
    last_err = None
    for attempt in range(3):
        try:
            res = run_bass_kernel_spmd(nc, in_maps, list(range(N_CORES)), trace=trace,
                                       tmpdir=tmpdir)
            break
        except Exception as e:  # noqa: BLE001
            last_err = e
            if attempt == 2:
                raise
            import time as _time
            _time.sleep(20)
    outs = np.concatenate([np.asarray(res.results[i]["out"]) for i in range(N_CORES)], axis=0)
    B, S = condition.shape[0], condition.shape[1]
    Pfull = outs.reshape(B, S, cfg.D)
    H = cfg.D // 2
    return (Pfull[..., :H], Pfull[..., H:]), res


def kernel(condition, w_gate, w_down, norm_weight, w_out):
    (scale, shift), _ = run(condition, w_gate, w_down, norm_weight, w_out)
    return scale, shift


# revision 70
# speedup vs baseline: 1.2019x; 1.0016x over previous
"""Trainium2 Bass kernel for nn_AdaLNConditioning (HGRNBitMLP + AdaLN head).

Strategy:
- Data-parallel over tokens: 8192 tokens -> 1024 per core, no collectives.
- Host precomputes ternary weight quantization (BitNet b1.58 global-mean
  scale) and packs transposed weight tiles in streaming order as fp8e4
  (ternary {-1,0,1} is exact in e4m3; PE mixed bf16 x fp8 matmul verified
  bit-exact on HW). Halves weight HBM traffic and DGE descriptor count
  vs bf16.
- On device, per token tile [128, D]: RMSNorm stats + per-token int8
  quantization (round-to-nearest-even via the 1.5*2^23 magic constant,
  bit-exact with jnp.round), quantized codes stored as bf16 (integers
  <= 127 are exact in bf16), transposed into [K, token] layout with PE
  transposes batched 8-per-PSUM-bank + one DVE copy-out in the prologue
  (no DGE dependency) and DMA-xbar transposes on the sync queue in
  steady state.
- Matmuls run on bf16 codes x fp8 weights with f32 PSUM accumulation ->
  exact integer arithmetic; per-token dequant scale applied at PSUM
  evacuation (fused into ScalarE/VectorE copy). The bf16 N=512 matmul
  stream is the PE roofline for this problem: fp8 DoubleRow fails
  accuracy (e4m3 activations ~2.8%/layer vs the 2e-2 budget; an exact
  hi/lo split costs 2 DoubleRow matmuls = 1.39x bf16), and uint8 matmul
  is rejected by walrus codegen (s3d3_mm_dtype ISA check).
- Each layer is processed in NP=4 pieces of TP=2 token tiles (rather
  than 2 halves of 4): the first piece's quant is the only exposed
  (PE-idle) latency, so halving the piece cuts the prologue roughly in
  half. Weights stream once per piece (2x the traffic of the half
  schedule) which still fits the DMA budget when split across queues.
- swiglu intermediate z and down-proj output h round-trip through HBM
  in f32 (bf16 storage costs ~1.5e-2 rel err; f32 keeps e2e ~1.4e-3).
- DMA queues (only SP/Activation/gpsimd can initiate DMAs): sync DGE =
  code transposes only (the scalar-queue xbar path corrupts data, so
  sync is the only valid transpose queue); scalar DGE = weight
  half-chunk 0 + activation reads (x/z/h); gpsimd DGE = weight
  half-chunk 1 + z/h/out writes. Prologue piece 0 splits its x loads
  across sync+scalar (both otherwise idle there).
- norm_weight is all-ones for this module (checked on host): the AdaLN
  RMSNorm folds into the L3 bit_linear renorm as a pure [P,1] scalar
  chain on the existing ssq stats; no nw broadcast or second stats
  pass. A general nw path is kept as fallback.
"""

import sys
from contextlib import ExitStack

import numpy as np
import ml_dtypes

sys.path.insert(0, "/opt/trn_rl_repo")

import concourse.bass as bass  # noqa: E402
import concourse.tile as tile  # noqa: E402
from concourse import bacc  # noqa: E402
from concourse import mybir  # noqa: E402
from concourse.masks import make_identity  # noqa: E402

AF = mybir.ActivationFunctionType
ALU = mybir.AluOpType
F32 = mybir.dt.float32
BF16 = mybir.dt.bfloat16
FP8 = mybir.dt.float8e4

P = 128
MAGIC = 12582912.0  # 1.5 * 2**23: add+store rounds f32 to nearest-even integer
N_CORES = 8


class Cfg:
    def __init__(self, T=1024, D=4096, INTER=4096, CW=512, TP=2):
        self.T = T            # tokens per core
        self.D = D            # model dim (k of L1/L3, out of L2/L3)
        self.INTER = INTER    # swiglu intermediate
        self.CW = CW          # output-chunk width (matmul moving free dim)
        self.TT = T // P      # token tiles per core
        self.TP = TP          # token tiles per piece
        self.NP = self.TT // TP
        self.GCH = 2 * INTER // CW  # L1 chunks (v/gate interleaved)
        self.DCH = D // CW          # L2/L3 chunks
        self.KT1 = D // P
        self.KT2 = INTER // P


def host_weight_quant(w):
    """BitNet ternary quant. Returns (codes {-1,0,1} f32, scale) matching
    jnp: scale = 1/clip(mean|w|, 1e-5); q = clip(round(w*scale), -1, 1)."""
    mean_abs = np.mean(np.abs(w), dtype=np.float64).astype(np.float32)
    s = np.float32(1.0) / np.maximum(mean_abs, np.float32(1e-5))
    q = np.clip(np.round(w * s), -1, 1).astype(np.float32)
    return q, s


def pack_weight(WqT, col_starts, cfg):
    """Pack WqT [K, O] into [n_chunks, 2, P, KT/2, CW] fp8 half-chunk
    streaming layout: per chunk, two DMA-able halves, each with 8KB of
    contiguous per-partition data (k-tile-major within the half)."""
    K = WqT.shape[0]
    KT = K // P
    KH = KT // 2
    out = np.empty((len(col_starts), 2, P, KH, cfg.CW), dtype=ml_dtypes.float8_e4m3fn)
    for ci, c0 in enumerate(col_starts):
        blk = WqT[:, c0:c0 + cfg.CW]                       # [K, CW]
        # row k = kt*P + p -> [hc, p, kh, cw] with kt = hc*KH + kh
        blk = blk.reshape(2, KH, P, cfg.CW).transpose(0, 2, 1, 3)
        out[ci] = blk.astype(ml_dtypes.float8_e4m3fn)
    return out


def build_nc(cfg, sg, sd, so, nw_ones=True):
    """Build the single-core (SPMD) Bass program."""
    nc = bacc.Bacc()
    T, D, INTER, CW = cfg.T, cfg.D, cfg.INTER, cfg.CW
    TP, NP = cfg.TP, cfg.NP
    KT1, KT2, GCH, DCH = cfg.KT1, cfg.KT2, cfg.GCH, cfg.DCH
    QW = min(1024, D)             # quant sub-chunk width
    KTQ = QW // P                 # k-tiles per quant sub-chunk (psum slab)

    x_p = nc.declare_dram_parameter("x", [T, D], F32, isOutput=False)
    wg_p = nc.declare_dram_parameter("wg", [GCH, 2, P, KT1 // 2, CW], FP8, isOutput=False)
    wd_p = nc.declare_dram_parameter("wd", [DCH, 2, P, KT2 // 2, CW], FP8, isOutput=False)
    wo_p = nc.declare_dram_parameter("wo", [DCH, 2, P, KT1 // 2, CW], FP8, isOutput=False)
    if not nw_ones:
        nw_p = nc.declare_dram_parameter("nw", [1, D], F32, isOutput=False)
    out_p = nc.declare_dram_parameter("out", [T, D], F32, isOutput=True)

    c_gate = float(1.0 / (127.0 * sg))
    c_down = float(1.0 / (127.0 * sd))
    c_out = float(1.0 / (127.0 * so))

    with ExitStack() as ctx:
        tc = ctx.enter_context(tile.TileContext(nc))
        singles = ctx.enter_context(tc.tile_pool(name="singles", bufs=1))
        small = ctx.enter_context(tc.tile_pool(name="small", bufs=96))
        xin = ctx.enter_context(tc.tile_pool(name="xin", bufs=2))      # [P,QW] f32
        rts = ctx.enter_context(tc.tile_pool(name="rts", bufs=4))      # [P,QW] f32 scratch
        qt_pool = ctx.enter_context(tc.tile_pool(name="qt", bufs=2))   # [P,KT,TP*P] bf16
        wpool = ctx.enter_context(tc.tile_pool(name="wpool", bufs=8))  # [P,KT/2,CW] fp8
        gv = ctx.enter_context(tc.tile_pool(name="gv", bufs=2))        # [P,TP,CW] f32 per tag
        zpool = ctx.enter_context(tc.tile_pool(name="zpool", bufs=2))  # [P,TP,CW] f32
        mm_ps = ctx.enter_context(tc.tile_pool(name="mmps", bufs=6, space="PSUM"))
        tp_ps = ctx.enter_context(tc.tile_pool(name="tpps", bufs=2, space="PSUM"))
        dram = ctx.enter_context(tc.tile_pool(name="dram", bufs=1, space="DRAM"))

        eps_t = {}
        for ev in (1e-8, 1e-6):
            et = singles.tile([P, 1], F32, name=f"eps{ev}")
            nc.vector.memset(et, ev)
            eps_t[ev] = et
        ident = singles.tile([P, P], BF16, name="ident")
        make_identity(nc, ident)
        if not nw_ones:
            nw_bc = singles.tile([P, D], F32)
            nw_ap = nw_p[:]
            nc.sync.dma_start(
                out=nw_bc,
                in_=bass.AP(tensor=nw_ap.tensor, offset=nw_ap.offset, ap=[[0, P], [1, D]]),
            )

        def reduce_cols(parts, fn):
            """Combine [P,1] tiles with a binary DVE op; returns final tile."""
            while len(parts) > 1:
                nxt = []
                for i in range(0, len(parts) - 1, 2):
                    o = small.tile([P, 1], F32, tag="s", name="comb")
                    fn(o, parts[i], parts[i + 1])
                    nxt.append(o)
                if len(parts) % 2:
                    nxt.append(parts[-1])
                parts = nxt
            return parts[0]

        def quant_gen(src_ap, KTn, base_tt, qTt, c_t, c_const, eps, nw=False,
                      nw_eps=None, pe_tp=False):
            """Norm + int8-quant + transpose for TP token tiles of one piece.

            Quant scale is qs = 127/absmax(t2) (the rsqrt factor cancels
            algebraically between quant and dequant); the dequant scale
            c = (am*c_const)*r carries the norm factor r off the critical
            path. rt = (t2*qs + MAGIC) rounds to integer+MAGIC at the f32
            store; the -MAGIC subtract produces bf16 codes (exact for ints
            <= 127), then transposes move them into [K, token] layout.

            nw path: reference computes h1 = h*rsqrt(mean h^2 + nw_eps)*nw,
            then bit_linear renorms h2 = h1*rsqrt(mean h1^2 + eps); both
            fold into one per-token factor r = r1*r2 on (h*nw).  With
            nw == 1 the codes equal the plain path's and
            mean((h*r1)^2) == r1^2*ssq/DL, so the second elementwise stats
            pass collapses into [P,1] ops on ssq.
            """
            DL = KTn * P
            NQ = DL // QW
            for i in range(TP):
                tt = base_tt + i
                x_js = []
                for j in range(NQ):
                    x_j = xin.tile([P, QW], F32, tag="xin", bufs=10, name="xj")
                    # prologue: sync+scalar queues are otherwise idle, so
                    # split the x bolus across them; steady state: scalar
                    # queue (transposes own sync, writes own gpsimd)
                    if pe_tp:
                        eng = nc.sync if j % 2 == 1 else nc.scalar
                    else:
                        eng = nc.scalar
                    eng.dma_start(
                        out=x_j,
                        in_=src_ap[tt * P:(tt + 1) * P, j * QW:(j + 1) * QW])
                    x_js.append(x_j)
                sparts = []
                for j in range(NQ):
                    so_ = rts.tile([P, QW], F32, tag="sq", bufs=2)
                    ssj = small.tile([P, 1], F32, tag="s", name="ssj")
                    nc.scalar.activation(so_, x_js[j], AF.Square,
                                         accum_out=ssj)
                    sparts.append(ssj)
                ssq = reduce_cols(sparts, nc.vector.tensor_add)
                r = small.tile([P, 1], F32, tag="s")
                aparts = []
                if not nw:
                    std = small.tile([P, 1], F32, tag="s")
                    nc.scalar.activation(std, ssq, AF.Sqrt, scale=1.0 / DL, bias=eps_t[eps])
                    nc.vector.reciprocal(r, std)
                    for j in range(NQ):
                        amj = small.tile([P, 1], F32, tag="s", name="amj")
                        nc.vector.tensor_reduce(amj, x_js[j],
                                                axis=mybir.AxisListType.X, op=ALU.max,
                                                apply_absolute_value=True)
                        aparts.append(amj)
                else:
                    std1 = small.tile([P, 1], F32, tag="s")
                    nc.scalar.activation(std1, ssq, AF.Sqrt, scale=1.0 / DL,
                                         bias=eps_t[nw_eps])
                    r1 = small.tile([P, 1], F32, tag="s")
                    nc.vector.reciprocal(r1, std1)
                    if nw_ones:
                        ssq2 = ssq
                        for j in range(NQ):
                            amj = small.tile([P, 1], F32, tag="s", name="amj")
                            nc.vector.tensor_reduce(amj, x_js[j],
                                                    axis=mybir.AxisListType.X,
                                                    op=ALU.max,
                                                    apply_absolute_value=True)
                            aparts.append(amj)
                    else:
                        s2parts = []
                        for j in range(NQ):
                            t2j = rts.tile([P, QW], F32, tag="t2", bufs=1)
                            nc.vector.tensor_mul(t2j, x_js[j],
                                                 nw_bc[:, j * QW:(j + 1) * QW])
                            so2 = rts.tile([P, QW], F32, tag="sq", bufs=1)
                            ss2j = small.tile([P, 1], F32, tag="s", name="ss2j")
                            nc.scalar.activation(so2, t2j, AF.Square, accum_out=ss2j)
                            s2parts.append(ss2j)
                            amj = small.tile([P, 1], F32, tag="s", name="amj")
                            nc.vector.tensor_reduce(amj, t2j, axis=mybir.AxisListType.X,
                                                    op=ALU.max, apply_absolute_value=True)
                            aparts.append(amj)
                        ssq2 = reduce_cols(s2parts, nc.vector.tensor_add)
                    u = small.tile([P, 1], F32, tag="s")
                    nc.vector.tensor_mul(u, r1, r1)
                    w2 = small.tile([P, 1], F32, tag="s")
                    nc.vector.tensor_mul(w2, u, ssq2)
                    std2 = small.tile([P, 1], F32, tag="s")
                    nc.scalar.activation(std2, w2, AF.Sqrt, scale=1.0 / DL, bias=eps_t[eps])
                    r2 = small.tile([P, 1], F32, tag="s")
                    nc.vector.reciprocal(r2, std2)
                    nc.vector.tensor_mul(r, r1, r2)
                am = reduce_cols(aparts, nc.vector.tensor_max)
                invam = small.tile([P, 1], F32, tag="s")
                nc.vector.reciprocal(invam, am)
                qs = small.tile([P, 1], F32, tag="s")
                nc.vector.tensor_scalar_mul(qs, invam, 127.0)
                nc.vector.scalar_tensor_tensor(c_t[:, i:i + 1], am, c_const, r,
                                               op0=ALU.mult, op1=ALU.mult)
                # rt = t2*qs + MAGIC (f32 store rounds to nearest-even int)
                rtjs = []
                for j in range(NQ):
                    if not nw or nw_ones:
                        src_j = x_js[j]
                    else:
                        src_j = rts.tile([P, QW], F32, tag="t2", bufs=1)
                        nc.vector.tensor_mul(src_j, x_js[j],
                                             nw_bc[:, j * QW:(j + 1) * QW])
                    rt_j = rts.tile([P, QW], F32, tag="rt", bufs=4)
                    nc.vector.tensor_scalar(rt_j, src_j, scalar1=qs, scalar2=MAGIC,
                                            op0=ALU.mult, op1=ALU.add)
                    rtjs.append(rt_j)
                # subtract MAGIC -> bf16 codes, then transpose into qT.
                # Prologue piece (pe_tp): PE transposes batched 8-per-PSUM
                # bank + one DVE copy-out (short latency chain, no DGE).
                # Steady-state pieces: DMA-xbar transposes on the sync DGE
                # queue, which they get to themselves (the scalar-queue
                # xbar path corrupts data, so sync is the only valid
                # transpose queue).
                # qb stays on vector for BOTH parities: a scalar-engine qb
                # waits on the transpose that frees its buffer, and that
                # head-of-line blocks evac ACTIVATEs for 10s of us at piece
                # boundaries. bufs=8 reaches a full piece back so the
                # recycle wait is always long-satisfied.
                for j in range(NQ):
                    q_j = rts.tile([P, QW], BF16, tag="qb", bufs=8)
                    nc.vector.tensor_scalar_add(q_j, rtjs[j], -MAGIC)
                    dst = qTt[:, j * KTQ:(j + 1) * KTQ, i * P:(i + 1) * P]
                    # PE transposes only for the very first token tile: tile
                    # 1's PE transposes would sit in the PE FIFO ahead of the
                    # first product matmuls (which only need tile 0's codes)
                    # and idle the PE ~10us; tile 1 rides the sync xbar and
                    # overlaps the first matmul stream instead.
                    if pe_tp and i == 0:
                        tp = tp_ps.tile([P, KTQ, P], BF16, tag="tp")
                        for b in range(KTQ):
                            nc.tensor.transpose(
                                tp[:, b, :], q_j[:, b * P:(b + 1) * P], ident
                            )
                        nc.vector.tensor_copy(dst, tp)
                    else:
                        nc.sync.dma_start_transpose(dst, q_j)
                yield

        def mm_gen(w_p, nch, KTn, qTt, evac):
            KH = KTn // 2
            ii = list(range(TP))
            for c in range(nch):
                wts = []
                for g in range(2):
                    wt = wpool.tile([P, KH, CW], FP8, tag="w")
                    weng = nc.scalar if g == 0 else nc.gpsimd
                    weng.dma_start(out=wt, in_=w_p[c, g])
                    wts.append(wt)
                for i in ii:
                    ps = mm_ps.tile([P, CW], F32, tag="mm")
                    for kt in range(KTn):
                        rhs = wts[kt // KH][:, kt % KH, :]
                        nc.tensor.matmul(
                            ps,
                            lhsT=qTt[:, kt, i * P:(i + 1) * P],
                            rhs=rhs,
                            start=(kt == 0),
                            stop=(kt == KTn - 1),
                        )
                    evac.step(c, i, ps)  # psum bank freed per i-group
                evac.flush(c, ii)
                yield

        # ---- dram intermediates (per piece: no false cross-piece deps) ----
        z_ds = [dram.tile([T // NP, INTER], F32, name=f"z{p}", tag=f"z{p}")
                for p in range(NP)]
        z_rs = [zd[:].rearrange("(a p) n -> p a n", p=P) for zd in z_ds]
        h_ds = [dram.tile([T // NP, D], F32, name=f"h{p}", tag=f"h{p}")
                for p in range(NP)]
        h_rs = [hd[:].rearrange("(a p) n -> p a n", p=P) for hd in h_ds]
        out_r = out_p[:].rearrange("(a p) n -> p a n", p=P)
        st = {}

        class Evac1:
            # chunk order v0,g0,v1,g1,...: v dequantized x c1^2 (extra c1
            # pre-applies gate dequant); z = (psum_g * sigmoid(psum_g*c1)) * v
            def __init__(self, c1h, piece):
                self.c1h = c1h
                self.piece = piece

            def step(self, c, i, ps):
                cc = self.c1h[:, i:i + 1]
                if c % 2 == 0:
                    if st.get("v_new", True):
                        st["v"] = gv.tile([P, TP, CW], F32, tag="v", name="v_t")
                        st["v_new"] = False
                    nc.vector.tensor_scalar(st["v"][:, i, :], ps, scalar1=cc,
                                            scalar2=cc, op0=ALU.mult, op1=ALU.mult)
                else:
                    if st.get("sig_new", True):
                        st["sig"] = gv.tile([P, TP, CW], F32, tag="sig", bufs=1, name="sig_t")
                        st["z"] = zpool.tile([P, TP, CW], F32, tag="z", name="z_t")
                        st["sig_new"] = False
                    nc.scalar.activation(st["sig"][:, i, :], ps, AF.Sigmoid, scale=cc)
                    nc.vector.tensor_mul(st["z"][:, i, :], ps, st["sig"][:, i, :])

            def flush(self, c, ii):
                if c % 2 == 0:
                    st["sig_new"] = True
                else:
                    st["v_new"] = True
                    lo, hi = ii[0], ii[-1] + 1
                    z_t = st["z"]
                    nc.vector.tensor_mul(z_t[:, lo:hi, :], z_t[:, lo:hi, :],
                                         st["v"][:, lo:hi, :])
                    nc.gpsimd.dma_start(
                        out=z_rs[self.piece][:, lo:hi, (c // 2) * CW:(c // 2 + 1) * CW],
                        in_=z_t[:, lo:hi, :]
                    )

        class EvacPlain:
            def __init__(self, c_th, dst_r, row0, final=False):
                self.c_th = c_th
                self.dst_r = dst_r
                self.row0 = row0
                self.final = final

            def step(self, c, i, ps):
                if st.get("o_new", True):
                    st["o"] = zpool.tile([P, TP, CW], F32, tag="z", name="o_t")
                    st["o_new"] = False
                cc = self.c_th[:, i:i + 1]
                if i % 2 == 0:
                    nc.vector.tensor_scalar(st["o"][:, i, :], ps, scalar1=cc,
                                            scalar2=None, op0=ALU.mult)
                else:
                    nc.scalar.activation(st["o"][:, i, :], ps, AF.Copy, scale=cc)

            def flush(self, c, ii):
                st["o_new"] = True
                if self.final:
                    # last piece: per-i writes on alternating queues so the
                    # post-last-matmul DMA drain is ~1 write, not a backlog
                    for i in ii:
                        eng = nc.gpsimd if i % 2 == 0 else nc.sync
                        eng.dma_start(
                            out=self.dst_r[:, self.row0 + i:self.row0 + i + 1,
                                           c * CW:(c + 1) * CW],
                            in_=st["o"][:, i:i + 1, :]
                        )
                    return
                lo, hi = ii[0], ii[-1] + 1
                row0 = self.row0 + lo
                nc.gpsimd.dma_start(
                    out=self.dst_r[:, row0:row0 + hi - lo, c * CW:(c + 1) * CW],
                    in_=st["o"][:, lo:hi, :]
                )

        # ---- pipelined stages: emission INTERLEAVED so quant(stage k+1)
        # overlaps mm(stage k) on every engine's instruction stream ----
        def stage_factory(L, p):
            def mk():
                ct = singles.tile([P, TP], F32, name=f"c{L}_{p}")
                if L == 1:
                    qT = qt_pool.tile([P, KT1, TP * P], BF16, tag="qt")
                    qg = quant_gen(x_p[:], KT1, p * TP, qT, ct, c_gate, 1e-8,
                                   pe_tp=(p == 0))
                    mmf = lambda: mm_gen(wg_p, GCH, KT1, qT, Evac1(ct, p))
                    return qg, mmf
                if L == 2:
                    qT = qt_pool.tile([P, KT2, TP * P], BF16, tag="qt")
                    qg = quant_gen(z_ds[p][:], KT2, 0, qT, ct, c_down, 1e-8)
                    mmf = lambda: mm_gen(wd_p, DCH, KT2, qT,
                                         EvacPlain(ct, h_rs[p], 0))
                    return qg, mmf
                qT = qt_pool.tile([P, KT1, TP * P], BF16, tag="qt")
                qg = quant_gen(h_ds[p][:], KT1, 0, qT, ct, c_out, 1e-8,
                               nw=True, nw_eps=1e-6)
                mmf = lambda: mm_gen(wo_p, DCH, KT1, qT,
                                     EvacPlain(ct, out_r, p * TP,
                                               final=(p == NP - 1)))
                return qg, mmf
            return mk

        stage_mks = [stage_factory(L, p) for L in (1, 2, 3) for p in range(NP)]
        qg0, mmf = stage_mks[0]()
        for _ in qg0:
            pass
        for k in range(len(stage_mks)):
            mm = mmf()
            if k + 1 < len(stage_mks):
                qn, mmf = stage_mks[k + 1]()
            else:
                qn = None
            # Emit the ENTIRE next-stage quant right after the first chunk:
            # gradual interleave leaves the quant tail sandwiched behind
            # psum-gated evac ops at the end of each engine FIFO, which
            # re-creates a ~13us late-quant equilibrium at every piece
            # boundary. Early full emission lets it run at piece start.
            # (A reader emitted before its writer does NOT get a dependency
            # — emission order defines the DAG — so the quant must be fully
            # emitted before any matmul that reads its codes.)
            ci = 0
            for _ in mm:
                ci += 1
                if qn is not None and ci >= 1:
                    for _ in qn:
                        pass
                    qn = None
    return nc


def prepare_inputs(condition, w_gate, w_down, norm_weight, w_out, cfg, n_cores=N_CORES):
    """Host-side: quantize+pack weights, shard tokens. Returns (in_maps, scales)."""
    TOK = condition.shape[0] * condition.shape[1]
    X = np.ascontiguousarray(condition.reshape(TOK, cfg.D).astype(np.float32, copy=False))

    Wg, sg = host_weight_quant(np.asarray(w_gate, dtype=np.float32))
    Wd, sd = host_weight_quant(np.asarray(w_down, dtype=np.float32))
    Wo, so = host_weight_quant(np.asarray(w_out, dtype=np.float32))

    # L1 chunk order interleaves v/gate so swiglu can fuse per chunk pair
    l1_cols = []
    for i in range(cfg.INTER // cfg.CW):
        l1_cols += [cfg.INTER + i * cfg.CW, i * cfg.CW]
    WG = pack_weight(Wg.T, l1_cols, cfg)
    WD = pack_weight(Wd.T, [i * cfg.CW for i in range(cfg.D // cfg.CW)], cfg)
    WO = pack_weight(Wo.T, [i * cfg.CW for i in range(cfg.D // cfg.CW)], cfg)

    nw = np.ascontiguousarray(np.asarray(norm_weight, dtype=np.float32).reshape(1, cfg.D))
    nw_ones = bool(np.all(nw == np.float32(1.0)))

    in_maps = []
    for i in range(n_cores):
        m = {
            "x": np.ascontiguousarray(X[i * cfg.T:(i + 1) * cfg.T]),
            "wg": WG, "wd": WD, "wo": WO,
        }
        if not nw_ones:
            m["nw"] = nw
        in_maps.append(m)
    return in_maps, (sg, sd, so), nw_ones


def run(condition, w_gate, w_down, norm_weight, w_out, cfg=None, trace=False, tmpdir=None):
    from concourse.bass_utils import run_bass_kernel_spmd
    if cfg is None:
        cfg = Cfg()
    in_maps, (sg, sd, so), nw_ones = prepare_inputs(condition, w_gate, w_down,
                                                     norm_weight, w_out, cfg)
    nc = build_nc(cfg, sg, sd, so, nw_ones=nw_ones)
    nc.finalize()
    # transient NRT_EXEC_UNIT_UNRECOVERABLE device crashes recover on retry
    last_err = None
    for attempt in range(3):
        try:
            res = run_bass_kernel_spmd(nc, in_maps, list(range(N_CORES)), trace=trace,
                                       tmpdir=tmpdir)
            break
        except Exception as e:  # noqa: BLE001
            last_err = e
            if attempt == 2:
                raise
            import time as _time
            _time.sleep(20)
    outs = np.concatenate([np.asarray(res.results[i]["out"]) for i in range(N_CORES)], axis=0)
    B, S = condition.shape[0], condition.shape[1]
    Pfull = outs.reshape(B, S, cfg.D)
    H = cfg.D // 2
    return (Pfull[..., :H], Pfull[..., H:]), res


def kernel(condition, w_gate, w_down, norm_weight, w_out):
    (scale, shift), _ = run(condition, w_gate, w_down, norm_weight, w_out)
    return scale, shift
